# revision 18
# baseline (speedup 1.0000x reference)
"""Trainium2 Bass kernel for the 4-kernel MMD permutation test (nn_DUAL_78237124264373).

Math (per core, 25 of the 200 permutations; everything else replicated):
  Z = [X; Y] (768 x 64), d2[r,c] = ||Z_r - Z_c||^2 built on the PE as a single
  rank-66 matmul  d2 = L^T R  with L = [Zt; sq; 1], R = [-2 Zt; 1; sq].
  K0_k = f_k(d2) (symmetric kernel matrix, no diag zeroing).
  With a_p the X-half indicator of permutation p and the zeroed-K statistics
  expressed through symmetric-K0 quantities plus corrections through
  e_j = K0[j, 384+j] (the zeroed stripe), every U_b entry reduces to
     U_b = kap*(q0 - arow0) + W_corr @ e_k + (2/c2)*t + C_k
  where q0 = a K0 a, arow0 = a K0 1 come from one matmul M0 = A_aug K0,
  t is the per-permutation paired-sample sum gathered element-wise from d2
  in DRAM (sentinel offsets handle zeroed-stripe pairs), and W_corr folds the
  three correction coefficients into one host-built {0,1}-combination matrix.
"""

import os
import sys

import numpy as np

if "/opt/trn_rl_repo" not in sys.path:
    sys.path.insert(0, "/opt/trn_rl_repo")

import concourse.bacc as bacc
import concourse.bass as bass
import concourse.mybir as mybir
import concourse.tile as tile
from concourse import bass_utils

N = 384
NM = 768
D = 64
NPER = 200
NC = 8
PPC = NPER // NC  # 25
C1 = float(N * (N - 1))
C2 = float(N * N)
KAP = np.float32(2.0 / C1 + 2.0 / C2)
CB1 = np.float32(1.0 / C1 + 2.0 / C2)
CB2 = np.float32(1.0 / C1)
TCO = np.float32(2.0 / C2)
IC1 = np.float32(1.0 / C1)
IC2 = np.float32(1.0 / C2)
LARGE = np.float32(1e9)
KERNELS = ("gaussian", "laplacian", "gaussian", "laplacian")

F32 = mybir.dt.float32
F32R = mybir.dt.float32r
I32 = mybir.dt.int32
AF = mybir.ActivationFunctionType
ALU = mybir.AluOpType


def _r(ap):
    return ap  # plain fp32 matmul (float32r needs f32r-rounded producers)


def _build():
    dbg_no_ind = bool(os.environ.get("DBG_NO_IND"))
    dbg_no_pulls = bool(os.environ.get("DBG_NO_PULLS"))
    dbg_no_stride = bool(os.environ.get("DBG_NO_STRIDE"))
    nc = bacc.Bacc("TRN2", target_bir_lowering=False, debug=False)
    with tile.TileContext(nc) as tc:
        with tc.tile_pool(name="dram", bufs=1, space="DRAM") as dram, \
             tc.tile_pool(name="io", bufs=1) as io, \
             tc.tile_pool(name="big", bufs=1) as big, \
             tc.tile_pool(name="kpool", bufs=6) as kpool, \
             tc.tile_pool(name="scr", bufs=2) as scr, \
             tc.tile_pool(name="sml", bufs=1) as sml:

            def din(name, shape, dt=F32):
                t = dram.tile(shape, dt, kind="ExternalInput", name=name,
                              uniquify=False)
                return t

            zt_d = din("zt", [D + 2, NM])  # rows 0-63 Zt, 64 zeros, 65 ones
            astk_d = din("astk", [27, NM])
            atp_d = din("atp", [128, 6 * 27])
            wct_d = din("wct", [128, 3 * PPC])
            fold_d = din("fold", [75, PPC])
            zp_d = din("zp", [9984, 128], mybir.dt.bfloat16)
            aux_d = din("aux", [128, 10])
            aux4_d = din("aux4", [4, 2])
            ident_d = din("ident", [128, 128])
            out_d = dram.tile([4, 1 + PPC], F32, kind="ExternalOutput",
                              name="out", uniquify=False)

            # ---- phase 0: input DMAs ----
            L_all = io.tile([D + 2, NM], F32, name="L_all")
            nc.sync.dma_start(out=L_all[:], in_=zt_d[:])
            astk = io.tile([27, NM], F32, name="astk_sb")
            nc.sync.dma_start(out=astk[:], in_=astk_d[:])
            atp = io.tile([128, 6 * 27], F32, name="atp_sb")
            nc.sync.dma_start(out=atp[:], in_=atp_d[:])
            wct = io.tile([128, 3 * PPC], F32, name="wct_sb")
            nc.sync.dma_start(out=wct[:], in_=wct_d[:])
            fold = io.tile([75, PPC], F32, name="fold_sb")
            nc.sync.dma_start(out=fold[:], in_=fold_d[:])
            zp = io.tile([128, 78, 128], mybir.dt.bfloat16, name="zp_sb")
            nc.sync.dma_start(
                out=zp[:],
                in_=zp_d[:].rearrange("(b p) d -> p b d", p=128))
            aux = io.tile([128, 10], F32, name="aux_sb")
            nc.sync.dma_start(out=aux[:], in_=aux_d[:])
            aux4 = io.tile([4, 2], F32, name="aux4_sb")
            nc.sync.dma_start(out=aux4[:], in_=aux4_d[:])
            ident = io.tile([128, 128], F32, name="ident_sb")
            nc.sync.dma_start(out=ident[:], in_=ident_d[:])

            ones = io.tile([128, 1], F32, name="ones_sb")
            nc.vector.memset(ones[:], 1.0)

            R_all = io.tile([D + 2, NM], F32, name="R_all")
            d2sb = big.tile([128, 6 * NM], F32, name="d2sb")
            dist = big.tile([128, 6 * NM], F32, name="dist_sb")
            M0sb = big.tile([27, 4 * NM], F32, name="M0sb")

            with tc.tile_pool(name="psA", bufs=2, space="PSUM") as psA:
                # ---- phase 1: sq row + L/R assembly ----
                zt2 = scr.tile([D, NM], F32, name="zt2", tag="zt2", bufs=1)
                nc.scalar.activation(zt2[:], L_all[0:D, :], AF.Square)
                ps_sq = psA.tile([1, NM], F32, name="ps_sq", tag="d2")
                nc.tensor.matmul(ps_sq[:, 0:512], _r(ones[0:D, 0:1]),
                                 _r(zt2[:, 0:512]), start=True, stop=True)
                nc.tensor.matmul(ps_sq[:, 512:NM], _r(ones[0:D, 0:1]),
                                 _r(zt2[:, 512:NM]), start=True, stop=True)
                sqrow = sml.tile([1, NM], F32, name="sqrow")
                nc.vector.tensor_copy(sqrow[:], ps_sq[:])
                # L = [Zt; sq; 1], R = [-2 Zt; 1; sq]; rows 64/65 go via DMA
                # (compute engines need 32-aligned partition bases).
                nc.sync.dma_start(out=L_all[D:D + 1, :], in_=sqrow[:])
                nc.vector.tensor_scalar_mul(R_all[0:D, :], L_all[0:D, :], -2.0)
                nc.sync.dma_start(out=R_all[D:D + 1, :],
                                  in_=L_all[D + 1:D + 2, :])
                nc.sync.dma_start(out=R_all[D + 1:D + 2, :], in_=sqrow[:])

                # ---- phase 2: d2 = L^T R, relu, spill to DRAM ----
                for r in range(6):
                    ps_d2 = psA.tile([128, NM], F32, name=f"ps_d2_{r}",
                                     tag="d2")
                    lhs = L_all[:, 128 * r:128 * (r + 1)]
                    nc.tensor.matmul(ps_d2[:, 0:512], _r(lhs),
                                     _r(R_all[:, 0:512]), start=True, stop=True)
                    nc.tensor.matmul(ps_d2[:, 512:NM], _r(lhs),
                                     _r(R_all[:, 512:NM]), start=True, stop=True)
                    sl = slice(NM * r, NM * (r + 1))
                    nc.scalar.activation(d2sb[:, sl], ps_d2[:], AF.Relu)

            # ---- phase 3/4: pair distances from host-arranged Z-row pairs ----
            # zp rows j<9600: (Z[pX], Z[pY]) for pair j=(p*384+i); rows
            # 9600..9983: the stripe pairs (Z[j], Z[384+j]) that give e_k.
            # d2all[p, b] = ||zx - zy||^2 lands t-pairs in cols 0..74 and
            # stripe pairs in cols 75..77 (9600 = 75*128).
            BF = mybir.dt.bfloat16
            pdiff = sml.tile([128, 78, 64], BF, name="pdiff")
            nc.vector.tensor_tensor(out=pdiff[:], in0=zp[:, :, 0:64],
                                    in1=zp[:, :, 64:128], op=ALU.subtract)
            pprod = sml.tile([128, 78, 64], BF, name="pprod")
            nc.vector.tensor_tensor(out=pprod[:], in0=pdiff[:], in1=pdiff[:],
                                    op=ALU.mult)
            d2all = sml.tile([128, 78], F32, name="d2all")
            nc.vector.tensor_reduce(d2all[:], pprod[:],
                                    axis=mybir.AxisListType.X, op=ALU.add)
            distp = sml.tile([128, 78], F32, name="distp")
            nc.scalar.activation(distp[:], d2all[:], AF.Sqrt, bias=aux[:, 8:9])
            e_all = sml.tile([128, 12], F32, name="e_all")  # col 4c+k
            tcol = sml.tile([PPC, 4], F32, name="tcol")

            # ---- phase 5: dist = sqrt(d2 + 1e-12) ----
            for r in range(6):
                sl = slice(NM * r, NM * (r + 1))
                nc.scalar.activation(dist[:, sl], d2sb[:, sl], AF.Sqrt,
                                     bias=aux[:, 8:9])

            arow = sml.tile([27, 4], F32, name="arow")
            colA = sml.tile([27, 4], F32, name="colA")
            q0c = sml.tile([27, 4], F32, name="q0c")

            with tc.tile_pool(name="psB", bufs=2, space="PSUM") as psB, \
                 tc.tile_pool(name="psC", bufs=1, space="PSUM") as psC:
                # per-kernel pair exps; t via column-sum matmul (9600=75*128
                # means perm p's pairs sit in psum partitions 3p..3p+2)
                ps_t = psC.tile([75, 4], F32, name="ps_t", tag="sm", bufs=3)
                for k, kern in enumerate(KERNELS):
                    psrc = d2all if kern == "gaussian" else distp
                    expk = scr.tile([128, 78], F32, name=f"expk{k}",
                                    tag="expk")
                    nc.scalar.activation(expk[:], psrc[:], AF.Exp,
                                         scale=aux[:, 2 * k:2 * k + 1],
                                         bias=aux[:, 2 * k + 1:2 * k + 2])
                    nc.vector.tensor_copy(e_all[:, k:12:4], expk[:, 75:78])
                    nc.tensor.matmul(ps_t[:, k:k + 1], expk[:, 0:75],
                                     ones[:, 0:1], start=True, stop=True)
                t75s = sml.tile([75, 4], F32, name="t75s")
                nc.vector.tensor_copy(t75s[:], ps_t[:])
                ps_tc = psC.tile([PPC, 4], F32, name="ps_tc", tag="sm", bufs=3)
                nc.tensor.matmul(ps_tc[:], fold[:], t75s[:],
                                 start=True, stop=True)
                nc.vector.tensor_copy(tcol[:], ps_tc[:])

                # ---- phase 6: K0_k = f_k(d2); M0 = A_aug @ K0; row stats ----
                for k, kern in enumerate(KERNELS):
                    src = d2sb if kern == "gaussian" else dist
                    ktiles = []
                    for c in range(6):
                        kt = kpool.tile([128, NM], F32, name=f"kt{k}_{c}",
                                        tag="kt")
                        sl = slice(NM * c, NM * (c + 1))
                        nc.scalar.activation(kt[:], src[:, sl], AF.Exp,
                                             scale=aux[:, 2 * k:2 * k + 1],
                                             bias=aux[:, 2 * k + 1:2 * k + 2])
                        ktiles.append(kt)
                    ps_m = psB.tile([27, NM], F32, name=f"ps_m{k}", tag="m0")
                    for c in range(6):
                        lhs = atp[:, 27 * c:27 * (c + 1)]
                        nc.tensor.matmul(ps_m[:, 0:512], _r(lhs),
                                         _r(ktiles[c][:, 0:512]),
                                         start=(c == 0), stop=(c == 5))
                        nc.tensor.matmul(ps_m[:, 512:NM], _r(lhs),
                                         _r(ktiles[c][:, 512:NM]),
                                         start=(c == 0), stop=(c == 5))
                    sl = slice(NM * k, NM * (k + 1))
                    nc.vector.tensor_scalar(
                        out=M0sb[:, sl], in0=ps_m[:], scalar1=1.0, scalar2=0.0,
                        op0=ALU.mult, op1=ALU.add,
                        accum_out=arow[:, k:k + 1])
                    sA = scr.tile([27, N], F32, name=f"sA{k}", tag="sA")
                    nc.vector.tensor_scalar(
                        out=sA[:], in0=M0sb[:, NM * k:NM * k + N], scalar1=1.0,
                        scalar2=0.0, op0=ALU.mult, op1=ALU.add,
                        accum_out=colA[:, k:k + 1])
                    sB = scr.tile([27, NM], F32, name=f"sB{k}", tag="sB")
                    nc.vector.tensor_tensor(out=sB[:], in0=M0sb[:, sl],
                                            in1=astk[:], op=ALU.mult)
                    nc.vector.tensor_reduce(q0c[:, k:k + 1], sB[:],
                                            axis=mybir.AxisListType.X,
                                            op=ALU.add)

                # ---- phase 7: corrections and stripe sums ----
                ps_corr = psC.tile([PPC, 4], F32, name="ps_corr", tag="sm", bufs=3)
                for c in range(3):
                    nc.tensor.matmul(ps_corr[:],
                                     wct[:, PPC * c:PPC * (c + 1)],
                                     e_all[:, 4 * c:4 * (c + 1)],
                                     start=(c == 0), stop=(c == 2))
                ps_se = psC.tile([12, 1], F32, name="ps_se", tag="sm", bufs=3)
                nc.tensor.matmul(ps_se[:], e_all[:], ones[:, 0:1],
                                 start=True, stop=True)
                sesum = sml.tile([12, 1], F32, name="sesum")
                nc.vector.tensor_copy(sesum[:], ps_se[:])

                # ---- phase 8: U_b assembly ----
                colB = sml.tile([27, 4], F32, name="colB")
                nc.vector.tensor_tensor(out=colB[:], in0=arow[:], in1=colA[:],
                                        op=ALU.subtract)
                t1 = sml.tile([27, 4], F32, name="t1")
                nc.vector.tensor_tensor(out=t1[:], in0=q0c[:], in1=arow[:],
                                        op=ALU.subtract)
                ub1 = sml.tile([PPC, 4], F32, name="ub1")
                nc.vector.tensor_scalar_mul(ub1[:], t1[0:PPC, :], float(KAP))
                nc.vector.tensor_tensor(out=ub1[:], in0=ub1[:], in1=ps_corr[:],
                                        op=ALU.add)
                tt = sml.tile([PPC, 4], F32, name="tt")
                nc.vector.tensor_scalar_mul(tt[:], tcol[:], float(TCO))
                nc.vector.tensor_tensor(out=ub1[:], in0=ub1[:], in1=tt[:],
                                        op=ALU.add)
                ps_ubt = psC.tile([4, PPC], F32, name="ps_ubt", tag="sm", bufs=3)
                nc.tensor.transpose(ps_ubt[:], ub1[:], ident[0:PPC, 0:PPC])

                # ---- phase 9: scalar pulls ----
                XX = sml.tile([4, 1], F32, name="XX")
                YX = sml.tile([4, 1], F32, name="YX")
                XY0 = sml.tile([4, 1], F32, name="XY0")
                YY = sml.tile([4, 1], F32, name="YY")
                se3 = sml.tile([4, 3], F32, name="se3")
                if dbg_no_pulls:
                    for t_ in (XX, YX, XY0, YY):
                        nc.vector.memset(t_[:], 1.0)
                    nc.vector.memset(se3[:], 1.0)
                else:
                    nc.sync.dma_start(out=XX[:], in_=colA[25:26, 0:4])
                    nc.sync.dma_start(out=YX[:], in_=colA[26:27, 0:4])
                    nc.sync.dma_start(out=XY0[:], in_=colB[25:26, 0:4])
                    nc.sync.dma_start(out=YY[:], in_=colB[26:27, 0:4])
                    for c in range(3):
                        nc.sync.dma_start(out=se3[0:4, c:c + 1],
                                          in_=sesum[4 * c:4 * c + 4, 0:1])

                # ---- phase 10: final output tile ----
                F = sml.tile([4, 1 + PPC], F32, name="Fout")
                se = sml.tile([4, 1], F32, name="se")
                nc.vector.tensor_reduce(se[:], se3[:], axis=mybir.AxisListType.X,
                                        op=ALU.add)
                s0t = sml.tile([4, 1], F32, name="s0t")
                nc.vector.tensor_tensor(out=s0t[:], in0=XX[:], in1=YX[:],
                                        op=ALU.add)
                nc.vector.tensor_tensor(out=s0t[:], in0=s0t[:], in1=XY0[:],
                                        op=ALU.add)
                nc.vector.tensor_tensor(out=s0t[:], in0=s0t[:], in1=YY[:],
                                        op=ALU.add)
                ck = sml.tile([4, 1], F32, name="ck")
                nc.vector.tensor_tensor(out=ck[:], in0=s0t[:], in1=se[:],
                                        op=ALU.subtract)
                nc.vector.tensor_tensor(out=ck[:], in0=ck[:],
                                        in1=aux4[:, 0:1], op=ALU.subtract)
                nc.vector.tensor_scalar_mul(ck[:], ck[:], float(IC1))
                nc.vector.tensor_scalar(
                    out=F[:, 1:1 + PPC], in0=ps_ubt[:], scalar1=ck[:],
                    scalar2=None, op0=ALU.add)
                u1 = sml.tile([4, 1], F32, name="u1")
                nc.vector.tensor_tensor(out=u1[:], in0=XX[:], in1=YY[:],
                                        op=ALU.add)
                nc.vector.tensor_tensor(out=u1[:], in0=u1[:],
                                        in1=aux4[:, 0:1], op=ALU.subtract)
                nc.vector.tensor_scalar_mul(u1[:], u1[:], float(IC1))
                u2 = sml.tile([4, 1], F32, name="u2")
                nc.vector.tensor_tensor(out=u2[:], in0=XY0[:], in1=se[:],
                                        op=ALU.subtract)
                nc.vector.tensor_scalar_mul(u2[:], u2[:], float(2.0 * IC2))
                nc.vector.tensor_tensor(out=F[:, 0:1], in0=u1[:], in1=u2[:],
                                        op=ALU.subtract)
                nc.sync.dma_start(out=out_d[:], in_=F[:])

    nc.compile()
    return nc


def _host_prep(X, Y, bandwidths, perms):
    X = np.ascontiguousarray(X, np.float32)
    Y = np.ascontiguousarray(Y, np.float32)
    perms = np.ascontiguousarray(perms, np.int32)
    Zt = np.zeros((D + 2, NM), np.float32)  # rows 0-63 Zt, 64 zeros, 65 ones
    Zt[0:D] = np.concatenate([X, Y], 0).T
    Zt[D + 1] = 1.0
    b = np.asarray(bandwidths, np.float64)
    gs = (-1.0 / (b * b)).astype(np.float32)
    gb = (gs.astype(np.float64) * 1e-12).astype(np.float32)
    ls = (-1.0 / b).astype(np.float32)
    aux = np.zeros((128, 10), np.float32)
    aux[:, 8] = 1e-12
    d0c = np.zeros(4, np.float64)
    for k, kern in enumerate(KERNELS):
        if kern == "gaussian":
            aux[:, 2 * k] = gs[k]
            aux[:, 2 * k + 1] = gb[k]
            d0c[k] = np.exp(-1e-12 / (b[k] * b[k]))
        else:
            aux[:, 2 * k] = ls[k]
            aux[:, 2 * k + 1] = 0.0
            d0c[k] = np.exp(-np.sqrt(1e-12) / b[k])
    aux4 = np.zeros((4, 2), np.float32)
    aux4[:, 0] = (768.0 * d0c).astype(np.float32)
    ident = np.eye(128, dtype=np.float32)
    foldm = (np.arange(75)[:, None] // 3 == np.arange(PPC)[None, :]
             ).astype(np.float32)

    shared = dict(zt=Zt, aux=aux, aux4=aux4, ident=ident, fold=foldm)
    maps = []
    for cid in range(NC):
        pm = perms[cid * PPC:(cid + 1) * PPC]
        A = np.zeros((27, NM), np.float32)
        A[np.arange(PPC)[:, None], pm[:, :N]] = 1
        A[25, :N] = 1
        A[26, N:] = 1
        atp = np.zeros((128, 6 * 27), np.float32)
        for c in range(6):
            atp[:, 27 * c:27 * (c + 1)] = A[:, 128 * c:128 * (c + 1)].T
        A1 = A[:PPC, :N]
        A2 = A[:PPC, N:]
        Wc = (-KAP * (A1 * A2) + CB1 * A1 + CB2 * A2).astype(np.float32)
        wct = np.zeros((128, 3 * PPC), np.float32)
        for c in range(3):
            wct[:, PPC * c:PPC * (c + 1)] = Wc[:, 128 * c:128 * (c + 1)].T
        pX = pm[:, :N].astype(np.int64).ravel()
        pY = pm[:, N:].astype(np.int64).ravel()
        # Pair-arranged Z rows: [zx | zy] per pair; stripe pairs (pY==pX+384)
        # get a sentinel row whose distance is huge so f_k -> 0, matching the
        # zeroed K stripe. Rows 9600..9983 are the stripe-diagonal pairs
        # themselves (they produce the e_k correction vectors).
        Zf = np.concatenate([X, Y], 0)                 # [768, 64] fp32
        zx = Zf[pX]
        zy = Zf[pY]
        stripe = pY == pX + N
        zx[stripe] = 0.0
        zy[stripe] = 0.0
        zx[stripe, 0] = 1e6  # d2=1e12: exp(-1e12/b^2)=exp(-1e6/b)=0, sqrt-safe
        j = np.arange(N)
        zp = np.concatenate([
            np.concatenate([zx, zy], 1),               # [9600, 128]
            np.concatenate([Zf[j], Zf[N + j]], 1),     # [384, 128]
        ], 0)
        import ml_dtypes
        zp = zp.astype(ml_dtypes.bfloat16)
        maps.append(dict(shared, astk=A, atp=atp, wct=wct, zp=zp))
    return maps


_NC_CACHE = None


def _get_nc():
    global _NC_CACHE
    if _NC_CACHE is None:
        _NC_CACHE = _build()
    return _NC_CACHE


def kernel(X, Y, bandwidths, perms):
    nc = _get_nc()
    in_maps = _host_prep(X, Y, bandwidths, perms)
    res = bass_utils.run_bass_kernel_spmd(nc, in_maps, list(range(NC)))
    full = np.zeros((4, 1 + NPER), np.float32)
    full[:, 0] = res.results[0]["out"][:, 0]
    for cid in range(NC):
        full[:, 1 + cid * PPC:1 + (cid + 1) * PPC] = res.results[cid]["out"][:, 1:]
    return full


# revision 30
# speedup vs baseline: 1.0841x; 1.0841x over previous
"""Trainium2 Bass kernel for the 4-kernel MMD permutation test (nn_DUAL_78237124264373).

Math (per core, 25 of the 200 permutations; everything else replicated):
  Z = [X; Y] (768 x 64), d2[r,c] = ||Z_r - Z_c||^2 built on the PE as a single
  rank-66 matmul  d2 = L^T R  with L = [Zt; sq; 1], R = [-2 Zt; 1; sq].
  K0_k = f_k(d2) (symmetric kernel matrix, no diag zeroing).
  With a_p the X-half indicator of permutation p and the zeroed-K statistics
  expressed through symmetric-K0 quantities plus corrections through
  e_j = K0[j, 384+j] (the zeroed stripe), every U_b entry reduces to
     U_b = kap*(q0 - arow0) + W_corr @ e_k + (2/c2)*t + C_k
  where q0 = a K0 a, arow0 = a K0 1 come from one matmul M0 = A_aug K0,
  t is the per-permutation paired-sample sum computed from host-arranged
  Z-row pairs (sentinel rows handle zeroed-stripe pairs), and W_corr folds
  the three correction coefficients into one host-built matrix.

Layout: the four kernels are column-tiled onto PE col-groups, so all
per-permutation statistics live at partition 32*k + p (kernel k, perm p) and
the DVE reductions run once over 128 partitions instead of 4x over 27.
"""

import os
import sys

import numpy as np

if "/opt/trn_rl_repo" not in sys.path:
    sys.path.insert(0, "/opt/trn_rl_repo")

import concourse.bacc as bacc
import concourse.bass as bass
import concourse.mybir as mybir
import concourse.tile as tile
from concourse import bass_utils

N = 384
NM = 768
D = 64
NPER = 200
NC = 8
PPC = NPER // NC  # 25
C1 = float(N * (N - 1))
C2 = float(N * N)
KAP = np.float32(2.0 / C1 + 2.0 / C2)
CB1 = np.float32(1.0 / C1 + 2.0 / C2)
CB2 = np.float32(1.0 / C1)
TCO = np.float32(2.0 / C2)
IC1 = np.float32(1.0 / C1)
IC2 = np.float32(1.0 / C2)
KERNELS = ("gaussian", "laplacian", "gaussian", "laplacian")

F32 = mybir.dt.float32
F32R = mybir.dt.float32r
BF16 = mybir.dt.bfloat16
AF = mybir.ActivationFunctionType
ALU = mybir.AluOpType


def _build():
    nc = bacc.Bacc("TRN2", target_bir_lowering=False, debug=False)
    with tile.TileContext(nc) as tc:
        with tc.tile_pool(name="dram", bufs=1, space="DRAM") as dram, \
             tc.tile_pool(name="io", bufs=1) as io, \
             tc.tile_pool(name="big", bufs=1) as big, \
             tc.tile_pool(name="kpool", bufs=4) as kpool, \
             tc.tile_pool(name="scr", bufs=2) as scr, \
             tc.tile_pool(name="sml", bufs=1) as sml:

            def din(name, shape, dt=F32):
                return dram.tile(shape, dt, kind="ExternalInput", name=name,
                                 uniquify=False)

            # One fused input tensor (single DMA); column layout below.
            W_IN = 2002
            bigin_d = din("bigin", [128, W_IN])
            zp_d = din("zp", [128, 9984], BF16)  # pair-arranged Z rows
            out_d = dram.tile([4, 1 + PPC], F32, kind="ExternalOutput",
                              name="out", uniquify=False)

            # ---- phase 0: input DMAs ----
            bigin = io.tile([128, W_IN], F32, name="bigin_sb")
            nc.sync.dma_start(out=bigin[:], in_=bigin_d[:])
            Lbig = bigin[0:D + 1, 0:NM]          # [Zt rows 0-63; ones row 64]
            astk = bigin[:, NM:2 * NM]           # A_aug rows at 32k+i
            atp = bigin[:, 1536:1536 + 192]      # A_aug^T chunks (32-padded)
            wct = bigin[:, 1728:1728 + 96]       # W_corr^T chunks (32-padded)
            fold = bigin[0:75, 1824:1824 + 32]   # 3->1 fold (32-padded)
            ident = bigin[:, 1856:1856 + 128]
            aux = bigin[:, 1984:1994]
            aux4 = bigin[0:1, 1994:2002]
            zpf = io.tile([128, 78 * 128], BF16, name="zp_sb")
            nc.gpsimd.dma_start(out=zpf[:], in_=zp_d[:])
            zp = zpf[:].rearrange("p (b d) -> p b d", d=128)

            ones = io.tile([128, 1], F32, name="ones_sb")
            nc.vector.memset(ones[:], 1.0)

            R_all = io.tile([D + 1, NM], F32, name="R_all")
            d2sb = big.tile([128, 6 * NM], F32, name="d2sb")
            dist = big.tile([128, 6 * NM], F32, name="dist_sb")
            M0sb = big.tile([128, NM], F32, name="M0sb")

            with tc.tile_pool(name="psA", bufs=2, space="PSUM") as psA:
                # ---- phase 1: sq = rowsums of Zt^2, landed at psum
                # partitions 0 (for the sq_col transposes) and 64 (for the
                # R matrix row) via col-tiling ----
                zt2 = scr.tile([D, NM], F32, name="zt2", tag="zt2", bufs=1)
                nc.vector.tensor_tensor(out=zt2[:], in0=Lbig[0:D, :],
                                        in1=Lbig[0:D, :], op=ALU.mult)
                ps_sq = psA.tile([128, NM], F32, name="ps_sq", tag="d2")
                for s in (slice(0, 512), slice(512, NM)):
                    nc.tensor.matmul(ps_sq[0:1, s], ones[0:D, 0:1], zt2[:, s],
                                     start=True, stop=True,
                                     skip_group_check=True)
                    nc.tensor.matmul(ps_sq[D:D + 1, s], ones[0:D, 0:1],
                                     zt2[:, s], start=True, stop=True,
                                     tile_position=(0, D),
                                     skip_group_check=True)
                sqrow = sml.tile([1, NM], F32, name="sqrow")
                nc.vector.tensor_copy(sqrow[:], ps_sq[0:1, :])
                # R = [-2 Zt; sq]; row 64 copies within partition 64.
                nc.vector.tensor_scalar_mul(R_all[0:D, :], Lbig[0:D, :], -2.0)
                nc.vector.tensor_copy(R_all[D:D + 1, :], ps_sq[D:D + 1, :])
                # sq as columns (for the relu bias): 6 tiny PE transposes
                ps_sqc = psA.tile([128, 8], F32, name="ps_sqc", tag="sqc",
                                  bufs=1)
                for r in range(6):
                    nc.tensor.matmul(ps_sqc[:, r:r + 1],
                                     sqrow[0:1, 128 * r:128 * (r + 1)],
                                     ones[0:1, 0:1], is_transpose=True,
                                     start=True, stop=True,
                                     skip_group_check=True)
                sqc = sml.tile([128, 8], F32, name="sqc")
                nc.vector.tensor_copy(sqc[:, 0:6], ps_sqc[:, 0:6])

                # ---- phase 2: psum = -2 Z Z^T + sq[c]; then
                # d2 = max(psum + sq[r], 0) on the DVE ----
                for r in range(6):
                    ps_d2 = psA.tile([128, NM], F32, name=f"ps_d2_{r}",
                                     tag="d2")
                    lhs = Lbig[:, 128 * r:128 * (r + 1)]
                    nc.tensor.matmul(ps_d2[:, 0:512], lhs, R_all[:, 0:512],
                                     start=True, stop=True)
                    nc.tensor.matmul(ps_d2[:, 512:NM], lhs, R_all[:, 512:NM],
                                     start=True, stop=True)
                    sl = slice(NM * r, NM * (r + 1))
                    nc.vector.tensor_scalar(
                        out=d2sb[:, sl], in0=ps_d2[:],
                        scalar1=sqc[:, r:r + 1], scalar2=0.0,
                        op0=ALU.add, op1=ALU.max)

            # ---- phase 3: pair distances from host-arranged Z-row pairs ----
            # zp rows j<9600: (Z[pX], Z[pY]) for pair j=(p*384+i); rows
            # 9600..9983 are the stripe pairs (Z[j], Z[384+j]) -> e_k.
            # 9600 = 75*128, so t-pairs land in cols 0..74 of d2all and the
            # stripe pairs in cols 75..77; perm p owns psum partitions 3p+c.
            pdiff = sml.tile([128, 78, 64], BF16, name="pdiff")
            nc.vector.tensor_tensor(out=pdiff[:], in0=zp[:, :, 0:64],
                                    in1=zp[:, :, 64:128], op=ALU.subtract)
            pprod = sml.tile([128, 78, 64], BF16, name="pprod")
            nc.vector.tensor_tensor(out=pprod[:], in0=pdiff[:], in1=pdiff[:],
                                    op=ALU.mult)
            d2all = sml.tile([128, 78], F32, name="d2all")
            nc.vector.tensor_reduce(d2all[:], pprod[:],
                                    axis=mybir.AxisListType.X, op=ALU.add)
            distp = sml.tile([128, 78], F32, name="distp")
            nc.scalar.activation(distp[:], d2all[:], AF.Sqrt, bias=aux[:, 8:9])

            # ---- phase 4: dist = sqrt(d2 + 1e-12) (one wide ACT) ----
            nc.scalar.activation(dist[:], d2sb[:], AF.Sqrt, bias=aux[:, 8:9])

            arow = sml.tile([128, 1], F32, name="arow")
            colA = sml.tile([128, 1], F32, name="colA")
            q0c = sml.tile([128, 1], F32, name="q0c")

            with tc.tile_pool(name="psB", bufs=1, space="PSUM") as psB, \
                 tc.tile_pool(name="psC", bufs=1, space="PSUM") as psC:
                # pair exps; t_k via column-sum matmul then a fold matmul
                # into partitions 32k+p
                ps_t = psC.tile([75, 4], F32, name="ps_t", tag="sm", bufs=3)
                expks = []
                for k, kern in enumerate(KERNELS):
                    psrc = d2all if kern == "gaussian" else distp
                    expk = scr.tile([128, 78], F32, name=f"expk{k}",
                                    tag="expk", bufs=4)
                    nc.scalar.activation(expk[:], psrc[:], AF.Exp,
                                         scale=aux[:, 2 * k:2 * k + 1],
                                         bias=aux[:, 2 * k + 1:2 * k + 2])
                    expks.append(expk)
                    nc.tensor.matmul(ps_t[:, k:k + 1], expk[:, 0:75],
                                     ones[:, 0:1], start=True, stop=True)
                t75s = sml.tile([75, 4], F32, name="t75s")
                nc.vector.tensor_copy(t75s[:], ps_t[:])
                ps_tc = psC.tile([128, 1], F32, name="ps_tc", tag="sm", bufs=3)
                for k in range(4):
                    nc.tensor.matmul(ps_tc[32 * k:32 * k + 32, 0:1], fold[:],
                                     t75s[:, k:k + 1], start=True, stop=True,
                                     tile_position=(0, 32 * k),
                                     skip_group_check=True)
                tcol = sml.tile([128, 1], F32, name="tcol")
                nc.vector.tensor_scalar_mul(tcol[:], ps_tc[:], float(TCO))

                # ---- phase 5: K0_k = f_k(d2); M0 = A_aug K0 col-tiled so
                # kernel k's rows land at partitions 32k+i ----
                kts = []
                for k, kern in enumerate(KERNELS):
                    src = d2sb if kern == "gaussian" else dist
                    kt = kpool.tile([128, 6 * NM], F32, name=f"kt{k}",
                                    tag="kt")
                    nc.scalar.activation(kt[:], src[:], AF.Exp,
                                         scale=aux[:, 2 * k:2 * k + 1],
                                         bias=aux[:, 2 * k + 1:2 * k + 2])
                    kts.append(kt)
                ps_m = psB.tile([128, NM], F32, name="ps_m")
                for c in range(6):
                    lhs = atp[:, 32 * c:32 * (c + 1)]
                    for k in range(4):
                        pr = slice(32 * k, 32 * k + 32)
                        nc.tensor.matmul(ps_m[pr, 0:512], lhs,
                                         kts[k][:, NM * c:NM * c + 512],
                                         start=(c == 0), stop=(c == 5),
                                         tile_position=(0, 32 * k),
                                         skip_group_check=True)
                        nc.tensor.matmul(ps_m[pr, 512:NM], lhs,
                                         kts[k][:, NM * c + 512:NM * (c + 1)],
                                         start=(c == 0), stop=(c == 5),
                                         tile_position=(0, 32 * k),
                                         skip_group_check=True)
                # row stats: copy+rowsum fused, first-half sum, masked q0
                nc.vector.tensor_scalar(
                    out=M0sb[:], in0=ps_m[:], scalar1=1.0, scalar2=0.0,
                    op0=ALU.mult, op1=ALU.add, accum_out=arow[:])
                sA = scr.tile([128, N], F32, name="sA", tag="sA")
                nc.vector.tensor_scalar(
                    out=sA[:], in0=M0sb[:, 0:N], scalar1=1.0, scalar2=0.0,
                    op0=ALU.mult, op1=ALU.add, accum_out=colA[:])
                sB = scr.tile([128, NM], F32, name="sB", tag="sB")
                nc.vector.tensor_tensor(out=sB[:], in0=M0sb[:], in1=astk[:],
                                        op=ALU.mult)
                nc.vector.tensor_reduce(q0c[:], sB[:],
                                        axis=mybir.AxisListType.X, op=ALU.add)

                # ---- phase 6: corrections (col-tiled) and stripe sums ----
                ps_corr = psC.tile([128, 1], F32, name="ps_corr", tag="sm",
                                   bufs=3)
                for c in range(3):
                    for k in range(4):
                        nc.tensor.matmul(
                            ps_corr[32 * k:32 * k + 32, 0:1],
                            wct[:, 32 * c:32 * (c + 1)],
                            expks[k][:, 75 + c:76 + c],
                            start=(c == 0), stop=(c == 2),
                            tile_position=(0, 32 * k),
                            skip_group_check=True)
                sesum = sml.tile([3, 4], F32, name="sesum")
                for k in range(4):
                    ps_sek = psC.tile([3, 1], F32, name=f"ps_se{k}", tag="se",
                                      bufs=2)
                    nc.tensor.matmul(ps_sek[:], expks[k][:, 75:78],
                                     ones[:, 0:1], start=True, stop=True)
                    nc.vector.tensor_copy(sesum[:, k:k + 1], ps_sek[:])

                # ---- phase 7: U_b assembly in the stacked [128,1] layout ----
                colB = sml.tile([128, 1], F32, name="colB")
                nc.vector.tensor_tensor(out=colB[:], in0=arow[:], in1=colA[:],
                                        op=ALU.subtract)
                ubv = sml.tile([128, 1], F32, name="ubv")
                nc.vector.tensor_tensor(out=ubv[:], in0=q0c[:], in1=arow[:],
                                        op=ALU.subtract)
                nc.vector.tensor_scalar_mul(ubv[:], ubv[:], float(KAP))
                nc.vector.tensor_tensor(out=ubv[:], in0=ubv[:], in1=ps_corr[:],
                                        op=ALU.add)
                nc.vector.tensor_tensor(out=ubv[:], in0=ubv[:], in1=tcol[:],
                                        op=ALU.add)
                # transpose [ubv | colA | colB] at once -> rows
                stk = sml.tile([128, 3], F32, name="stk")
                nc.vector.tensor_copy(stk[:, 0:1], ubv[:])
                nc.vector.tensor_copy(stk[:, 1:2], colA[:])
                nc.vector.tensor_copy(stk[:, 2:3], colB[:])
                ps_stk = psC.tile([3, 128], F32, name="ps_stk", tag="sm",
                                  bufs=3)
                nc.tensor.transpose(ps_stk[:], stk[:], ident[:])
                srow = sml.tile([3, 128], F32, name="srow")
                nc.vector.tensor_copy(srow[:], ps_stk[:])

                # ---- phase 8: fold everything into one partition-0 row ----
                # frow: [0:128)=ub, [128:256)=colA^T, [256:384)=colB^T,
                # [384:396)=sesum
                frow = sml.tile([1, 396], F32, name="frow")
                nc.sync.dma_start(out=frow[0:1, 0:384], in_=srow[:])
                nc.sync.dma_start(out=frow[0:1, 384:396], in_=sesum[:])

                def fr(base, step=32, count=4):
                    ap = frow[0:1, base:base + 1]
                    return bass.AP(ap.tensor, ap.offset,
                                   [ap.ap[0], [step, count]])

                XXv = fr(128 + 25)
                YXv = fr(128 + 26)
                XY0v = fr(256 + 25)
                YYv = fr(256 + 26)
                # se_k = sum_c sesum[4c+k]
                sev = sml.tile([1, 4], F32, name="sev")
                nc.vector.tensor_reduce(
                    sev[:],
                    frow[0:1, 384:396].rearrange("o (c k) -> o k c", k=4),
                    axis=mybir.AxisListType.X, op=ALU.add)
                s0t = sml.tile([1, 4], F32, name="s0t")
                nc.vector.tensor_tensor(out=s0t[:], in0=XXv, in1=YXv,
                                        op=ALU.add)
                nc.vector.tensor_tensor(out=s0t[:], in0=s0t[:], in1=XY0v,
                                        op=ALU.add)
                nc.vector.tensor_tensor(out=s0t[:], in0=s0t[:], in1=YYv,
                                        op=ALU.add)
                ck = sml.tile([1, 4], F32, name="ck")
                nc.vector.tensor_tensor(out=ck[:], in0=s0t[:], in1=sev[:],
                                        op=ALU.subtract)
                nc.vector.tensor_tensor(out=ck[:], in0=ck[:],
                                        in1=aux4[0:1, 0:4], op=ALU.subtract)
                nc.vector.tensor_scalar_mul(ck[:], ck[:], float(IC1))
                u1 = sml.tile([1, 4], F32, name="u1")
                nc.vector.tensor_tensor(out=u1[:], in0=XXv, in1=YYv,
                                        op=ALU.add)
                nc.vector.tensor_tensor(out=u1[:], in0=u1[:],
                                        in1=aux4[0:1, 0:4], op=ALU.subtract)
                nc.vector.tensor_scalar_mul(u1[:], u1[:], float(IC1))
                u2 = sml.tile([1, 4], F32, name="u2")
                nc.vector.tensor_tensor(out=u2[:], in0=XY0v, in1=sev[:],
                                        op=ALU.subtract)
                nc.vector.tensor_scalar_mul(u2[:], u2[:], float(2.0 * IC2))

                # ---- phase 9: contiguous U row + U_b block, two out DMAs ----
                uF = sml.tile([1, 4], F32, name="uF")
                nc.vector.tensor_tensor(out=uF[:], in0=u1[:], in1=u2[:],
                                        op=ALU.subtract)
                ubc = sml.tile([1, 4 * PPC], F32, name="ubc")
                ub_src = frow[0:1, 0:128].rearrange("o (k p) -> o k p", p=32)
                ckap = ck[0:1, 0:4]
                ck_b = bass.AP(ckap.tensor, ckap.offset,
                               [ckap.ap[0], [1, 4], [0, PPC]])
                nc.vector.tensor_tensor(
                    out=ubc[0:1, :].rearrange("o (k p) -> o k p", p=PPC),
                    in0=ub_src[0:1, :, 0:PPC], in1=ck_b, op=ALU.add)
                nc.sync.dma_start(
                    out=out_d[:, 0:1],
                    in_=uF[0:1, :].rearrange("o (k w) -> o k w", w=1))
                nc.sync.dma_start(
                    out=out_d[:, 1:1 + PPC],
                    in_=ubc[0:1, :].rearrange("o (k p) -> o k p", p=PPC))

    nc.compile()
    return nc


def _host_prep(X, Y, bandwidths, perms):
    X = np.ascontiguousarray(X, np.float32)
    Y = np.ascontiguousarray(Y, np.float32)
    perms = np.ascontiguousarray(perms, np.int32)
    Zt = np.zeros((D + 1, NM), np.float32)  # rows 0-63 Zt, row 64 ones
    Zt[0:D] = np.concatenate([X, Y], 0).T
    Zt[D] = 1.0
    b = np.asarray(bandwidths, np.float64)
    gs = (-1.0 / (b * b)).astype(np.float32)
    gb = (gs.astype(np.float64) * 1e-12).astype(np.float32)
    ls = (-1.0 / b).astype(np.float32)
    aux = np.zeros((128, 10), np.float32)
    aux[:, 8] = 1e-12
    d0c = np.zeros(4, np.float64)
    for k, kern in enumerate(KERNELS):
        if kern == "gaussian":
            aux[:, 2 * k] = gs[k]
            aux[:, 2 * k + 1] = gb[k]
            d0c[k] = np.exp(-1e-12 / (b[k] * b[k]))
        else:
            aux[:, 2 * k] = ls[k]
            aux[:, 2 * k + 1] = 0.0
            d0c[k] = np.exp(-np.sqrt(1e-12) / b[k])
    aux4 = np.zeros((1, 8), np.float32)
    aux4[0, 0:4] = (768.0 * d0c).astype(np.float32)
    ident = np.eye(128, dtype=np.float32)
    foldm = np.zeros((75, 32), np.float32)
    foldm[:, :PPC] = (np.arange(75)[:, None] // 3 ==
                      np.arange(PPC)[None, :])

    maps = []
    for cid in range(NC):
        pm = perms[cid * PPC:(cid + 1) * PPC]
        A = np.zeros((27, NM), np.float32)
        A[np.arange(PPC)[:, None], pm[:, :N]] = 1
        A[25, :N] = 1
        A[26, N:] = 1
        astk = np.zeros((128, NM), np.float32)
        for k in range(4):
            astk[32 * k:32 * k + 27] = A
        atp = np.zeros((128, 6 * 32), np.float32)
        for c in range(6):
            atp[:, 32 * c:32 * c + 27] = A[:, 128 * c:128 * (c + 1)].T
        A1 = A[:PPC, :N]
        A2 = A[:PPC, N:]
        Wc = (-KAP * (A1 * A2) + CB1 * A1 + CB2 * A2).astype(np.float32)
        wct = np.zeros((128, 3 * 32), np.float32)
        for c in range(3):
            wct[:, 32 * c:32 * c + PPC] = Wc[:, 128 * c:128 * (c + 1)].T
        pX = pm[:, :N].astype(np.int64).ravel()
        pY = pm[:, N:].astype(np.int64).ravel()
        # Pair-arranged Z rows: [zx | zy] per pair; stripe pairs (pY==pX+384)
        # get a sentinel row with huge distance so f_k -> 0 (matches the
        # zeroed K stripe). Rows 9600..9983 are the stripe-diagonal pairs
        # (they produce the e_k correction vectors).
        Zf = np.concatenate([X, Y], 0)
        zx = Zf[pX]
        zy = Zf[pY]
        stripe = pY == pX + N
        zx[stripe] = 0.0
        zy[stripe] = 0.0
        zx[stripe, 0] = 1e6  # d2=1e12: exp(-1e12/b^2)=exp(-1e6/b)=0
        j = np.arange(N)
        zp = np.concatenate([
            np.concatenate([zx, zy], 1),
            np.concatenate([Zf[j], Zf[N + j]], 1),
        ], 0)
        import ml_dtypes
        zp = zp.reshape(78, 128, 128).transpose(1, 0, 2).reshape(128, 9984)
        zp = np.ascontiguousarray(zp).astype(ml_dtypes.bfloat16)
        bigin = np.zeros((128, 2002), np.float32)
        bigin[0:D + 1, 0:NM] = Zt
        bigin[:, NM:2 * NM] = astk
        bigin[:, 1536:1536 + 192] = atp
        bigin[:, 1728:1728 + 96] = wct
        bigin[0:75, 1824:1824 + 32] = foldm
        bigin[:, 1856:1856 + 128] = ident
        bigin[:, 1984:1994] = aux
        bigin[0:1, 1994:2002] = aux4
        maps.append(dict(bigin=bigin, zp=zp))
    return maps


_NC_CACHE = None


def _get_nc():
    global _NC_CACHE
    if _NC_CACHE is None:
        _NC_CACHE = _build()
    return _NC_CACHE


def kernel(X, Y, bandwidths, perms):
    nc = _get_nc()
    in_maps = _host_prep(X, Y, bandwidths, perms)
    res = bass_utils.run_bass_kernel_spmd(nc, in_maps, list(range(NC)))
    full = np.zeros((4, 1 + NPER), np.float32)
    full[:, 0] = res.results[0]["out"][:, 0]
    for cid in range(NC):
        full[:, 1 + cid * PPC:1 + (cid + 1) * PPC] = \
            res.results[cid]["out"][:, 1:]
    return full


# revision 31
# speedup vs baseline: 1.1265x; 1.0391x over previous
"""Trainium2 Bass kernel for the 4-kernel MMD permutation test (nn_DUAL_78237124264373).

Math (per core, 25 of the 200 permutations; everything else replicated):
  Z = [X; Y] (768 x 64), d2[r,c] = ||Z_r - Z_c||^2 built on the PE as a single
  rank-66 matmul  d2 = L^T R  with L = [Zt; sq; 1], R = [-2 Zt; 1; sq].
  K0_k = f_k(d2) (symmetric kernel matrix, no diag zeroing).
  With a_p the X-half indicator of permutation p and the zeroed-K statistics
  expressed through symmetric-K0 quantities plus corrections through
  e_j = K0[j, 384+j] (the zeroed stripe), every U_b entry reduces to
     U_b = kap*(q0 - arow0) + W_corr @ e_k + (2/c2)*t + C_k
  where q0 = a K0 a, arow0 = a K0 1 come from one matmul M0 = A_aug K0,
  t is the per-permutation paired-sample sum computed from host-arranged
  Z-row pairs (sentinel rows handle zeroed-stripe pairs), and W_corr folds
  the three correction coefficients into one host-built matrix.

Layout: the four kernels are column-tiled onto PE col-groups, so all
per-permutation statistics live at partition 32*k + p (kernel k, perm p) and
the DVE reductions run once over 128 partitions instead of 4x over 27.
"""

import os
import sys

import numpy as np

if "/opt/trn_rl_repo" not in sys.path:
    sys.path.insert(0, "/opt/trn_rl_repo")

import concourse.bacc as bacc
import concourse.bass as bass
import concourse.mybir as mybir
import concourse.tile as tile
from concourse import bass_utils

N = 384
NM = 768
D = 64
NPER = 200
NC = 8
PPC = NPER // NC  # 25
C1 = float(N * (N - 1))
C2 = float(N * N)
KAP = np.float32(2.0 / C1 + 2.0 / C2)
CB1 = np.float32(1.0 / C1 + 2.0 / C2)
CB2 = np.float32(1.0 / C1)
TCO = np.float32(2.0 / C2)
IC1 = np.float32(1.0 / C1)
IC2 = np.float32(1.0 / C2)
KERNELS = ("gaussian", "laplacian", "gaussian", "laplacian")

F32 = mybir.dt.float32
F32R = mybir.dt.float32r
BF16 = mybir.dt.bfloat16
AF = mybir.ActivationFunctionType
ALU = mybir.AluOpType


def _build():
    nc = bacc.Bacc("TRN2", target_bir_lowering=False, debug=False)
    with tile.TileContext(nc) as tc:
        with tc.tile_pool(name="dram", bufs=1, space="DRAM") as dram, \
             tc.tile_pool(name="io", bufs=1) as io, \
             tc.tile_pool(name="big", bufs=1) as big, \
             tc.tile_pool(name="kpool", bufs=4) as kpool, \
             tc.tile_pool(name="scr", bufs=2) as scr, \
             tc.tile_pool(name="sml", bufs=1) as sml:

            def din(name, shape, dt=F32):
                return dram.tile(shape, dt, kind="ExternalInput", name=name,
                                 uniquify=False)

            # One fused input tensor (single DMA); column layout below.
            W_IN = 2002
            bigin_d = din("bigin", [128, W_IN])
            zp_d = din("zp", [128, 9984 + 192], BF16)  # pair rows + bf16 atp
            out_d = dram.tile([4, 1 + PPC], F32, kind="ExternalOutput",
                              name="out", uniquify=False)

            # ---- phase 0: input DMAs ----
            bigin = io.tile([128, W_IN], F32, name="bigin_sb")
            nc.sync.dma_start(out=bigin[:], in_=bigin_d[:])
            Lbig = bigin[0:D + 1, 0:NM]          # [Zt rows 0-63; ones row 64]
            astk = bigin[:, NM:2 * NM]           # A_aug rows at 32k+i
            atp = bigin[:, 1536:1536 + 192]      # A_aug^T chunks (32-padded)
            wct = bigin[:, 1728:1728 + 96]       # W_corr^T chunks (32-padded)
            fold = bigin[0:75, 1824:1824 + 32]   # 3->1 fold (32-padded)
            ident = bigin[:, 1856:1856 + 128]
            aux = bigin[:, 1984:1994]
            aux4 = bigin[0:1, 1994:2002]
            zpf = io.tile([128, 78 * 128 + 192], BF16, name="zp_sb")
            nc.gpsimd.dma_start(out=zpf[:], in_=zp_d[:])
            zp = zpf[:, 0:9984].rearrange("p (b d) -> p b d", d=128)
            atpb = zpf[:, 9984:9984 + 192]       # A_aug^T chunks in bf16

            ones = io.tile([128, 1], F32, name="ones_sb")
            nc.vector.memset(ones[:], 1.0)

            R_all = io.tile([D + 1, NM], F32, name="R_all")
            d2sb = big.tile([128, 6 * NM], F32, name="d2sb")
            dist = big.tile([128, 6 * NM], F32, name="dist_sb")
            M0sb = big.tile([128, NM], F32, name="M0sb")

            with tc.tile_pool(name="psA", bufs=2, space="PSUM") as psA:
                # ---- phase 1: sq = rowsums of Zt^2, landed at psum
                # partitions 0 (for the sq_col transposes) and 64 (for the
                # R matrix row) via col-tiling ----
                zt2 = scr.tile([D, NM], F32, name="zt2", tag="zt2", bufs=1)
                nc.vector.tensor_tensor(out=zt2[:], in0=Lbig[0:D, :],
                                        in1=Lbig[0:D, :], op=ALU.mult)
                ps_sq = psA.tile([128, NM], F32, name="ps_sq", tag="d2")
                for s in (slice(0, 512), slice(512, NM)):
                    nc.tensor.matmul(ps_sq[0:1, s], ones[0:D, 0:1], zt2[:, s],
                                     start=True, stop=True,
                                     skip_group_check=True)
                    nc.tensor.matmul(ps_sq[D:D + 1, s], ones[0:D, 0:1],
                                     zt2[:, s], start=True, stop=True,
                                     tile_position=(0, D),
                                     skip_group_check=True)
                sqrow = sml.tile([1, NM], F32, name="sqrow")
                nc.vector.tensor_copy(sqrow[:], ps_sq[0:1, :])
                # R = [-2 Zt; sq]; row 64 copies within partition 64.
                nc.vector.tensor_scalar_mul(R_all[0:D, :], Lbig[0:D, :], -2.0)
                nc.vector.tensor_copy(R_all[D:D + 1, :], ps_sq[D:D + 1, :])
                # sq as columns (for the relu bias): 6 tiny PE transposes
                ps_sqc = psA.tile([128, 8], F32, name="ps_sqc", tag="sqc",
                                  bufs=1)
                for r in range(6):
                    nc.tensor.matmul(ps_sqc[:, r:r + 1],
                                     sqrow[0:1, 128 * r:128 * (r + 1)],
                                     ones[0:1, 0:1], is_transpose=True,
                                     start=True, stop=True,
                                     skip_group_check=True)
                sqc = sml.tile([128, 8], F32, name="sqc")
                nc.vector.tensor_copy(sqc[:, 0:6], ps_sqc[:, 0:6])

                # ---- phase 2: psum = -2 Z Z^T + sq[c]; then
                # d2 = max(psum + sq[r], 0) on the DVE ----
                for r in range(6):
                    ps_d2 = psA.tile([128, NM], F32, name=f"ps_d2_{r}",
                                     tag="d2")
                    lhs = Lbig[:, 128 * r:128 * (r + 1)]
                    nc.tensor.matmul(ps_d2[:, 0:512], lhs, R_all[:, 0:512],
                                     start=True, stop=True)
                    nc.tensor.matmul(ps_d2[:, 512:NM], lhs, R_all[:, 512:NM],
                                     start=True, stop=True)
                    sl = slice(NM * r, NM * (r + 1))
                    nc.vector.tensor_scalar(
                        out=d2sb[:, sl], in0=ps_d2[:],
                        scalar1=sqc[:, r:r + 1], scalar2=0.0,
                        op0=ALU.add, op1=ALU.max)

            # ---- phase 3: pair distances from host-arranged Z-row pairs ----
            # zp rows j<9600: (Z[pX], Z[pY]) for pair j=(p*384+i); rows
            # 9600..9983 are the stripe pairs (Z[j], Z[384+j]) -> e_k.
            # 9600 = 75*128, so t-pairs land in cols 0..74 of d2all and the
            # stripe pairs in cols 75..77; perm p owns psum partitions 3p+c.
            pdiff = sml.tile([128, 78, 64], BF16, name="pdiff")
            nc.vector.tensor_tensor(out=pdiff[:], in0=zp[:, :, 0:64],
                                    in1=zp[:, :, 64:128], op=ALU.subtract)
            pprod = sml.tile([128, 78, 64], BF16, name="pprod")
            nc.vector.tensor_tensor(out=pprod[:], in0=pdiff[:], in1=pdiff[:],
                                    op=ALU.mult)
            d2all = sml.tile([128, 78], F32, name="d2all")
            nc.vector.tensor_reduce(d2all[:], pprod[:],
                                    axis=mybir.AxisListType.X, op=ALU.add)
            distp = sml.tile([128, 78], F32, name="distp")
            nc.scalar.activation(distp[:], d2all[:], AF.Sqrt, bias=aux[:, 8:9])

            # ---- phase 4: dist = sqrt(d2 + 1e-12) (one wide ACT) ----
            nc.scalar.activation(dist[:], d2sb[:], AF.Sqrt, bias=aux[:, 8:9])

            arow = sml.tile([128, 1], F32, name="arow")
            colA = sml.tile([128, 1], F32, name="colA")
            q0c = sml.tile([128, 1], F32, name="q0c")

            with tc.tile_pool(name="psB", bufs=1, space="PSUM") as psB, \
                 tc.tile_pool(name="psC", bufs=1, space="PSUM") as psC:
                # pair exps; t_k via column-sum matmul then a fold matmul
                # into partitions 32k+p
                ps_t = psC.tile([75, 4], F32, name="ps_t", tag="sm", bufs=3)
                expks = []
                for k, kern in enumerate(KERNELS):
                    psrc = d2all if kern == "gaussian" else distp
                    expk = scr.tile([128, 78], F32, name=f"expk{k}",
                                    tag="expk", bufs=4)
                    nc.scalar.activation(expk[:], psrc[:], AF.Exp,
                                         scale=aux[:, 2 * k:2 * k + 1],
                                         bias=aux[:, 2 * k + 1:2 * k + 2])
                    expks.append(expk)
                    nc.tensor.matmul(ps_t[:, k:k + 1], expk[:, 0:75],
                                     ones[:, 0:1], start=True, stop=True)
                t75s = sml.tile([75, 4], F32, name="t75s")
                nc.vector.tensor_copy(t75s[:], ps_t[:])
                ps_tc = psC.tile([128, 1], F32, name="ps_tc", tag="sm", bufs=3)
                for k in range(4):
                    nc.tensor.matmul(ps_tc[32 * k:32 * k + 32, 0:1], fold[:],
                                     t75s[:, k:k + 1], start=True, stop=True,
                                     tile_position=(0, 32 * k),
                                     skip_group_check=True)
                tcol = sml.tile([128, 1], F32, name="tcol")
                nc.vector.tensor_scalar_mul(tcol[:], ps_tc[:], float(TCO))

                # ---- phase 5: K0_k = f_k(d2); M0 = A_aug K0 col-tiled so
                # kernel k's rows land at partitions 32k+i ----
                kts = []
                for k, kern in enumerate(KERNELS):
                    src = d2sb if kern == "gaussian" else dist
                    kt = kpool.tile([128, 6 * NM], BF16, name=f"kt{k}",
                                    tag="kt")
                    nc.scalar.activation(kt[:], src[:], AF.Exp,
                                         scale=aux[:, 2 * k:2 * k + 1],
                                         bias=aux[:, 2 * k + 1:2 * k + 2])
                    kts.append(kt)
                ps_m = psB.tile([128, NM], F32, name="ps_m")
                for c in range(6):
                    lhs = atpb[:, 32 * c:32 * (c + 1)]
                    for k in range(4):
                        pr = slice(32 * k, 32 * k + 32)
                        nc.tensor.matmul(ps_m[pr, 0:512], lhs,
                                         kts[k][:, NM * c:NM * c + 512],
                                         start=(c == 0), stop=(c == 5),
                                         tile_position=(0, 32 * k),
                                         skip_group_check=True)
                        nc.tensor.matmul(ps_m[pr, 512:NM], lhs,
                                         kts[k][:, NM * c + 512:NM * (c + 1)],
                                         start=(c == 0), stop=(c == 5),
                                         tile_position=(0, 32 * k),
                                         skip_group_check=True)
                # row stats: copy+rowsum fused, first-half sum, masked q0
                nc.vector.tensor_scalar(
                    out=M0sb[:], in0=ps_m[:], scalar1=1.0, scalar2=0.0,
                    op0=ALU.mult, op1=ALU.add, accum_out=arow[:])
                sA = scr.tile([128, N], F32, name="sA", tag="sA")
                nc.vector.tensor_scalar(
                    out=sA[:], in0=M0sb[:, 0:N], scalar1=1.0, scalar2=0.0,
                    op0=ALU.mult, op1=ALU.add, accum_out=colA[:])
                sB = scr.tile([128, NM], F32, name="sB", tag="sB")
                nc.vector.tensor_tensor(out=sB[:], in0=M0sb[:], in1=astk[:],
                                        op=ALU.mult)
                nc.vector.tensor_reduce(q0c[:], sB[:],
                                        axis=mybir.AxisListType.X, op=ALU.add)

                # ---- phase 6: corrections (col-tiled) and stripe sums ----
                ps_corr = psC.tile([128, 1], F32, name="ps_corr", tag="sm",
                                   bufs=3)
                for c in range(3):
                    for k in range(4):
                        nc.tensor.matmul(
                            ps_corr[32 * k:32 * k + 32, 0:1],
                            wct[:, 32 * c:32 * (c + 1)],
                            expks[k][:, 75 + c:76 + c],
                            start=(c == 0), stop=(c == 2),
                            tile_position=(0, 32 * k),
                            skip_group_check=True)
                sesum = sml.tile([3, 4], F32, name="sesum")
                for k in range(4):
                    ps_sek = psC.tile([3, 1], F32, name=f"ps_se{k}", tag="se",
                                      bufs=2)
                    nc.tensor.matmul(ps_sek[:], expks[k][:, 75:78],
                                     ones[:, 0:1], start=True, stop=True)
                    nc.vector.tensor_copy(sesum[:, k:k + 1], ps_sek[:])

                # ---- phase 7: U_b assembly in the stacked [128,1] layout ----
                colB = sml.tile([128, 1], F32, name="colB")
                nc.vector.tensor_tensor(out=colB[:], in0=arow[:], in1=colA[:],
                                        op=ALU.subtract)
                ubv = sml.tile([128, 1], F32, name="ubv")
                nc.vector.tensor_tensor(out=ubv[:], in0=q0c[:], in1=arow[:],
                                        op=ALU.subtract)
                nc.vector.tensor_scalar_mul(ubv[:], ubv[:], float(KAP))
                nc.vector.tensor_tensor(out=ubv[:], in0=ubv[:], in1=ps_corr[:],
                                        op=ALU.add)
                nc.vector.tensor_tensor(out=ubv[:], in0=ubv[:], in1=tcol[:],
                                        op=ALU.add)
                # transpose [ubv | colA | colB] at once -> rows
                stk = sml.tile([128, 3], F32, name="stk")
                nc.vector.tensor_copy(stk[:, 0:1], ubv[:])
                nc.vector.tensor_copy(stk[:, 1:2], colA[:])
                nc.vector.tensor_copy(stk[:, 2:3], colB[:])
                ps_stk = psC.tile([3, 128], F32, name="ps_stk", tag="sm",
                                  bufs=3)
                nc.tensor.transpose(ps_stk[:], stk[:], ident[:])
                srow = sml.tile([3, 128], F32, name="srow")
                nc.vector.tensor_copy(srow[:], ps_stk[:])

                # ---- phase 8: fold everything into one partition-0 row ----
                # frow: [0:128)=ub, [128:256)=colA^T, [256:384)=colB^T,
                # [384:396)=sesum
                frow = sml.tile([1, 396], F32, name="frow")
                nc.sync.dma_start(out=frow[0:1, 0:384], in_=srow[:])
                nc.sync.dma_start(out=frow[0:1, 384:396], in_=sesum[:])

                def fr(base, step=32, count=4):
                    ap = frow[0:1, base:base + 1]
                    return bass.AP(ap.tensor, ap.offset,
                                   [ap.ap[0], [step, count]])

                XXv = fr(128 + 25)
                YXv = fr(128 + 26)
                XY0v = fr(256 + 25)
                YYv = fr(256 + 26)
                # se_k = sum_c sesum[4c+k]
                sev = sml.tile([1, 4], F32, name="sev")
                nc.vector.tensor_reduce(
                    sev[:],
                    frow[0:1, 384:396].rearrange("o (c k) -> o k c", k=4),
                    axis=mybir.AxisListType.X, op=ALU.add)
                s0t = sml.tile([1, 4], F32, name="s0t")
                nc.vector.tensor_tensor(out=s0t[:], in0=XXv, in1=YXv,
                                        op=ALU.add)
                nc.vector.tensor_tensor(out=s0t[:], in0=s0t[:], in1=XY0v,
                                        op=ALU.add)
                nc.vector.tensor_tensor(out=s0t[:], in0=s0t[:], in1=YYv,
                                        op=ALU.add)
                ck = sml.tile([1, 4], F32, name="ck")
                nc.vector.tensor_tensor(out=ck[:], in0=s0t[:], in1=sev[:],
                                        op=ALU.subtract)
                nc.vector.tensor_tensor(out=ck[:], in0=ck[:],
                                        in1=aux4[0:1, 0:4], op=ALU.subtract)
                nc.vector.tensor_scalar_mul(ck[:], ck[:], float(IC1))
                u1 = sml.tile([1, 4], F32, name="u1")
                nc.vector.tensor_tensor(out=u1[:], in0=XXv, in1=YYv,
                                        op=ALU.add)
                nc.vector.tensor_tensor(out=u1[:], in0=u1[:],
                                        in1=aux4[0:1, 0:4], op=ALU.subtract)
                nc.vector.tensor_scalar_mul(u1[:], u1[:], float(IC1))
                u2 = sml.tile([1, 4], F32, name="u2")
                nc.vector.tensor_tensor(out=u2[:], in0=XY0v, in1=sev[:],
                                        op=ALU.subtract)
                nc.vector.tensor_scalar_mul(u2[:], u2[:], float(2.0 * IC2))

                # ---- phase 9: contiguous U row + U_b block, two out DMAs ----
                uF = sml.tile([1, 4], F32, name="uF")
                nc.vector.tensor_tensor(out=uF[:], in0=u1[:], in1=u2[:],
                                        op=ALU.subtract)
                ubc = sml.tile([1, 4 * PPC], F32, name="ubc")
                ub_src = frow[0:1, 0:128].rearrange("o (k p) -> o k p", p=32)
                ckap = ck[0:1, 0:4]
                ck_b = bass.AP(ckap.tensor, ckap.offset,
                               [ckap.ap[0], [1, 4], [0, PPC]])
                nc.vector.tensor_tensor(
                    out=ubc[0:1, :].rearrange("o (k p) -> o k p", p=PPC),
                    in0=ub_src[0:1, :, 0:PPC], in1=ck_b, op=ALU.add)
                nc.sync.dma_start(
                    out=out_d[:, 0:1],
                    in_=uF[0:1, :].rearrange("o (k w) -> o k w", w=1))
                nc.sync.dma_start(
                    out=out_d[:, 1:1 + PPC],
                    in_=ubc[0:1, :].rearrange("o (k p) -> o k p", p=PPC))

    nc.compile()
    return nc


def _host_prep(X, Y, bandwidths, perms):
    X = np.ascontiguousarray(X, np.float32)
    Y = np.ascontiguousarray(Y, np.float32)
    perms = np.ascontiguousarray(perms, np.int32)
    Zt = np.zeros((D + 1, NM), np.float32)  # rows 0-63 Zt, row 64 ones
    Zt[0:D] = np.concatenate([X, Y], 0).T
    Zt[D] = 1.0
    b = np.asarray(bandwidths, np.float64)
    gs = (-1.0 / (b * b)).astype(np.float32)
    gb = (gs.astype(np.float64) * 1e-12).astype(np.float32)
    ls = (-1.0 / b).astype(np.float32)
    aux = np.zeros((128, 10), np.float32)
    aux[:, 8] = 1e-12
    d0c = np.zeros(4, np.float64)
    for k, kern in enumerate(KERNELS):
        if kern == "gaussian":
            aux[:, 2 * k] = gs[k]
            aux[:, 2 * k + 1] = gb[k]
            d0c[k] = np.exp(-1e-12 / (b[k] * b[k]))
        else:
            aux[:, 2 * k] = ls[k]
            aux[:, 2 * k + 1] = 0.0
            d0c[k] = np.exp(-np.sqrt(1e-12) / b[k])
    aux4 = np.zeros((1, 8), np.float32)
    aux4[0, 0:4] = (768.0 * d0c).astype(np.float32)
    ident = np.eye(128, dtype=np.float32)
    foldm = np.zeros((75, 32), np.float32)
    foldm[:, :PPC] = (np.arange(75)[:, None] // 3 ==
                      np.arange(PPC)[None, :])

    maps = []
    for cid in range(NC):
        pm = perms[cid * PPC:(cid + 1) * PPC]
        A = np.zeros((27, NM), np.float32)
        A[np.arange(PPC)[:, None], pm[:, :N]] = 1
        A[25, :N] = 1
        A[26, N:] = 1
        astk = np.zeros((128, NM), np.float32)
        for k in range(4):
            astk[32 * k:32 * k + 27] = A
        atp = np.zeros((128, 6 * 32), np.float32)
        for c in range(6):
            atp[:, 32 * c:32 * c + 27] = A[:, 128 * c:128 * (c + 1)].T
        A1 = A[:PPC, :N]
        A2 = A[:PPC, N:]
        Wc = (-KAP * (A1 * A2) + CB1 * A1 + CB2 * A2).astype(np.float32)
        wct = np.zeros((128, 3 * 32), np.float32)
        for c in range(3):
            wct[:, 32 * c:32 * c + PPC] = Wc[:, 128 * c:128 * (c + 1)].T
        pX = pm[:, :N].astype(np.int64).ravel()
        pY = pm[:, N:].astype(np.int64).ravel()
        # Pair-arranged Z rows: [zx | zy] per pair; stripe pairs (pY==pX+384)
        # get a sentinel row with huge distance so f_k -> 0 (matches the
        # zeroed K stripe). Rows 9600..9983 are the stripe-diagonal pairs
        # (they produce the e_k correction vectors).
        Zf = np.concatenate([X, Y], 0)
        zx = Zf[pX]
        zy = Zf[pY]
        stripe = pY == pX + N
        zx[stripe] = 0.0
        zy[stripe] = 0.0
        zx[stripe, 0] = 1e6  # d2=1e12: exp(-1e12/b^2)=exp(-1e6/b)=0
        j = np.arange(N)
        zp = np.concatenate([
            np.concatenate([zx, zy], 1),
            np.concatenate([Zf[j], Zf[N + j]], 1),
        ], 0)
        import ml_dtypes
        zp = zp.reshape(78, 128, 128).transpose(1, 0, 2).reshape(128, 9984)
        zp = np.concatenate([zp, atp], 1).astype(ml_dtypes.bfloat16)
        bigin = np.zeros((128, 2002), np.float32)
        bigin[0:D + 1, 0:NM] = Zt
        bigin[:, NM:2 * NM] = astk
        bigin[:, 1536:1536 + 192] = atp
        bigin[:, 1728:1728 + 96] = wct
        bigin[0:75, 1824:1824 + 32] = foldm
        bigin[:, 1856:1856 + 128] = ident
        bigin[:, 1984:1994] = aux
        bigin[0:1, 1994:2002] = aux4
        maps.append(dict(bigin=bigin, zp=zp))
    return maps


_NC_CACHE = None


def _get_nc():
    global _NC_CACHE
    if _NC_CACHE is None:
        _NC_CACHE = _build()
    return _NC_CACHE


def kernel(X, Y, bandwidths, perms):
    nc = _get_nc()
    in_maps = _host_prep(X, Y, bandwidths, perms)
    res = bass_utils.run_bass_kernel_spmd(nc, in_maps, list(range(NC)))
    full = np.zeros((4, 1 + NPER), np.float32)
    full[:, 0] = res.results[0]["out"][:, 0]
    for cid in range(NC):
        full[:, 1 + cid * PPC:1 + (cid + 1) * PPC] = \
            res.results[cid]["out"][:, 1:]
    return full


# revision 32
# speedup vs baseline: 1.2697x; 1.1271x over previous
"""Trainium2 Bass kernel for the 4-kernel MMD permutation test (nn_DUAL_78237124264373).

Math (per core, 25 of the 200 permutations; everything else replicated):
  Z = [X; Y] (768 x 64), d2[r,c] = ||Z_r - Z_c||^2 built on the PE as a single
  rank-66 matmul  d2 = L^T R  with L = [Zt; sq; 1], R = [-2 Zt; 1; sq].
  K0_k = f_k(d2) (symmetric kernel matrix, no diag zeroing).
  With a_p the X-half indicator of permutation p and the zeroed-K statistics
  expressed through symmetric-K0 quantities plus corrections through
  e_j = K0[j, 384+j] (the zeroed stripe), every U_b entry reduces to
     U_b = kap*(q0 - arow0) + W_corr @ e_k + (2/c2)*t + C_k
  where q0 = a K0 a, arow0 = a K0 1 come from one matmul M0 = A_aug K0,
  t is the per-permutation paired-sample sum computed from host-arranged
  Z-row pairs (sentinel rows handle zeroed-stripe pairs), and W_corr folds
  the three correction coefficients into one host-built matrix.

Layout: the four kernels are column-tiled onto PE col-groups, so all
per-permutation statistics live at partition 32*k + p (kernel k, perm p) and
the DVE reductions run once over 128 partitions instead of 4x over 27.
"""

import os
import sys

import numpy as np

if "/opt/trn_rl_repo" not in sys.path:
    sys.path.insert(0, "/opt/trn_rl_repo")

import concourse.bacc as bacc
import concourse.bass as bass
import concourse.mybir as mybir
import concourse.tile as tile
from concourse import bass_utils

N = 384
NM = 768
D = 64
NPER = 200
NC = 8
PPC = NPER // NC  # 25
C1 = float(N * (N - 1))
C2 = float(N * N)
KAP = np.float32(2.0 / C1 + 2.0 / C2)
CB1 = np.float32(1.0 / C1 + 2.0 / C2)
CB2 = np.float32(1.0 / C1)
TCO = np.float32(2.0 / C2)
IC1 = np.float32(1.0 / C1)
IC2 = np.float32(1.0 / C2)
KERNELS = ("gaussian", "laplacian", "gaussian", "laplacian")

F32 = mybir.dt.float32
F32R = mybir.dt.float32r
BF16 = mybir.dt.bfloat16
AF = mybir.ActivationFunctionType
ALU = mybir.AluOpType


def _build():
    nc = bacc.Bacc("TRN2", target_bir_lowering=False, debug=False)
    with tile.TileContext(nc) as tc:
        with tc.tile_pool(name="dram", bufs=1, space="DRAM") as dram, \
             tc.tile_pool(name="io", bufs=1) as io, \
             tc.tile_pool(name="big", bufs=1) as big, \
             tc.tile_pool(name="kpool", bufs=4) as kpool, \
             tc.tile_pool(name="scr", bufs=2) as scr, \
             tc.tile_pool(name="sml", bufs=1) as sml:

            def din(name, shape, dt=F32):
                return dram.tile(shape, dt, kind="ExternalInput", name=name,
                                 uniquify=False)

            # One fused input tensor (single DMA); column layout below.
            W_IN = 2002
            bigin_d = din("bigin", [128, W_IN])
            zp_d = din("zp", [128, 9984 + 192], BF16)  # pair rows + bf16 atp
            out_d = dram.tile([4, 1 + PPC], F32, kind="ExternalOutput",
                              name="out", uniquify=False)

            # ---- phase 0: input DMAs (Zt block first: it gates the PE) ----
            bigin = io.tile([128, W_IN], F32, name="bigin_sb")
            nc.sync.dma_start(out=bigin[:, 0:NM], in_=bigin_d[:, 0:NM])
            nc.sync.dma_start(out=bigin[:, NM:], in_=bigin_d[:, NM:])
            Lbig = bigin[0:D + 1, 0:NM]          # [Zt rows 0-63; ones row 64]
            astk = bigin[:, NM:2 * NM]           # A_aug rows at 32k+i
            atp = bigin[:, 1536:1536 + 192]      # A_aug^T chunks (32-padded)
            wct = bigin[:, 1728:1728 + 96]       # W_corr^T chunks (32-padded)
            fold = bigin[0:75, 1824:1824 + 32]   # 3->1 fold (32-padded)
            ident = bigin[:, 1856:1856 + 128]
            aux = bigin[:, 1984:1994]
            aux4 = bigin[0:1, 1994:2002]
            zpf = io.tile([128, 78 * 128 + 192], BF16, name="zp_sb")
            nc.sync.dma_start(out=zpf[:], in_=zp_d[:])
            zp = zpf[:, 0:9984].rearrange("p (b d) -> p b d", d=128)
            atpb = zpf[:, 9984:9984 + 192]       # A_aug^T chunks in bf16

            ones = io.tile([128, 1], F32, name="ones_sb")
            nc.vector.memset(ones[:], 1.0)

            R_all = io.tile([D + 1, NM], F32, name="R_all")
            d2sb = big.tile([128, 6 * NM], F32, name="d2sb")
            dist = big.tile([128, 6 * NM], F32, name="dist_sb")
            M0sb = big.tile([128, NM], F32, name="M0sb")

            with tc.tile_pool(name="psA", bufs=2, space="PSUM") as psA:
                # ---- phase 1: sq = rowsums of Zt^2, landed at psum
                # partitions 0 (for the sq_col transposes) and 64 (for the
                # R matrix row) via col-tiling ----
                zt2 = scr.tile([D, NM], F32, name="zt2", tag="zt2", bufs=1)
                nc.vector.tensor_tensor(out=zt2[:], in0=Lbig[0:D, :],
                                        in1=Lbig[0:D, :], op=ALU.mult)
                ps_sq = psA.tile([128, NM], F32, name="ps_sq", tag="d2")
                for s in (slice(0, 512), slice(512, NM)):
                    nc.tensor.matmul(ps_sq[0:1, s], ones[0:D, 0:1], zt2[:, s],
                                     start=True, stop=True,
                                     skip_group_check=True)
                    nc.tensor.matmul(ps_sq[D:D + 1, s], ones[0:D, 0:1],
                                     zt2[:, s], start=True, stop=True,
                                     tile_position=(0, D),
                                     skip_group_check=True)
                sqrow = sml.tile([1, NM], F32, name="sqrow")
                nc.vector.tensor_copy(sqrow[:], ps_sq[0:1, :])
                # R = [-2 Zt; sq]; row 64 copies within partition 64.
                nc.vector.tensor_scalar_mul(R_all[0:D, :], Lbig[0:D, :], -2.0)
                nc.vector.tensor_copy(R_all[D:D + 1, :], ps_sq[D:D + 1, :])
                # sq as columns (for the relu bias): 6 tiny PE transposes
                ps_sqc = psA.tile([128, 8], F32, name="ps_sqc", tag="sqc",
                                  bufs=1)
                for r in range(6):
                    nc.tensor.matmul(ps_sqc[:, r:r + 1],
                                     sqrow[0:1, 128 * r:128 * (r + 1)],
                                     ones[0:1, 0:1], is_transpose=True,
                                     start=True, stop=True,
                                     skip_group_check=True)
                sqc = sml.tile([128, 8], F32, name="sqc")
                nc.vector.tensor_copy(sqc[:, 0:6], ps_sqc[:, 0:6])

                # ---- phase 2: psum = -2 Z Z^T + sq[c]; then
                # d2 = max(psum + sq[r], 0) on the DVE ----
                for r in range(6):
                    ps_d2 = psA.tile([128, NM], F32, name=f"ps_d2_{r}",
                                     tag="d2")
                    lhs = Lbig[:, 128 * r:128 * (r + 1)]
                    nc.tensor.matmul(ps_d2[:, 0:512], lhs, R_all[:, 0:512],
                                     start=True, stop=True)
                    nc.tensor.matmul(ps_d2[:, 512:NM], lhs, R_all[:, 512:NM],
                                     start=True, stop=True)
                    sl = slice(NM * r, NM * (r + 1))
                    nc.vector.tensor_scalar(
                        out=d2sb[:, sl], in0=ps_d2[:],
                        scalar1=sqc[:, r:r + 1], scalar2=0.0,
                        op0=ALU.add, op1=ALU.max)

            # ---- phase 3: pair distances from host-arranged Z-row pairs ----
            # zp rows j<9600: (Z[pX], Z[pY]) for pair j=(p*384+i); rows
            # 9600..9983 are the stripe pairs (Z[j], Z[384+j]) -> e_k.
            # 9600 = 75*128, so t-pairs land in cols 0..74 of d2all and the
            # stripe pairs in cols 75..77; perm p owns psum partitions 3p+c.
            pdiff = sml.tile([128, 78, 64], BF16, name="pdiff")
            nc.vector.tensor_tensor(out=pdiff[:], in0=zp[:, :, 0:64],
                                    in1=zp[:, :, 64:128], op=ALU.subtract)
            pprod = sml.tile([128, 78, 64], BF16, name="pprod")
            nc.vector.tensor_tensor(out=pprod[:], in0=pdiff[:], in1=pdiff[:],
                                    op=ALU.mult)
            d2all = sml.tile([128, 78], F32, name="d2all")
            nc.vector.tensor_reduce(d2all[:], pprod[:],
                                    axis=mybir.AxisListType.X, op=ALU.add)
            distp = sml.tile([128, 78], F32, name="distp")

            # ---- phase 4: dist = sqrt(d2 + 1e-12) (one wide ACT) ----
            nc.scalar.activation(dist[:], d2sb[:], AF.Sqrt, bias=aux[:, 8:9])

            arow = sml.tile([128, 1], F32, name="arow")
            colA = sml.tile([128, 1], F32, name="colA")
            q0c = sml.tile([128, 1], F32, name="q0c")

            with tc.tile_pool(name="psB", bufs=1, space="PSUM") as psB, \
                 tc.tile_pool(name="psC", bufs=1, space="PSUM") as psC:
                # ---- phase 5: K0_k = f_k(d2); M0 = A_aug K0 col-tiled so
                # kernel k's rows land at partitions 32k+i ----
                kts = []
                for k, kern in enumerate(KERNELS):
                    src = d2sb if kern == "gaussian" else dist
                    kt = kpool.tile([128, 6 * NM], BF16, name=f"kt{k}",
                                    tag="kt")
                    nc.scalar.activation(kt[:], src[:], AF.Exp,
                                         scale=aux[:, 2 * k:2 * k + 1],
                                         bias=aux[:, 2 * k + 1:2 * k + 2])
                    kts.append(kt)
                ps_m = psB.tile([128, NM], F32, name="ps_m")
                for c in range(6):
                    lhs = atpb[:, 32 * c:32 * (c + 1)]
                    for k in range(4):
                        pr = slice(32 * k, 32 * k + 32)
                        nc.tensor.matmul(ps_m[pr, 0:512], lhs,
                                         kts[k][:, NM * c:NM * c + 512],
                                         start=(c == 0), stop=(c == 5),
                                         tile_position=(0, 32 * k),
                                         skip_group_check=True)
                        nc.tensor.matmul(ps_m[pr, 512:NM], lhs,
                                         kts[k][:, NM * c + 512:NM * (c + 1)],
                                         start=(c == 0), stop=(c == 5),
                                         tile_position=(0, 32 * k),
                                         skip_group_check=True)
                # row stats: copy+rowsum fused, first-half sum, masked q0
                nc.vector.tensor_scalar(
                    out=M0sb[:], in0=ps_m[:], scalar1=1.0, scalar2=0.0,
                    op0=ALU.mult, op1=ALU.add, accum_out=arow[:])
                sA = scr.tile([128, N], F32, name="sA", tag="sA")
                nc.vector.tensor_scalar(
                    out=sA[:], in0=M0sb[:, 0:N], scalar1=1.0, scalar2=0.0,
                    op0=ALU.mult, op1=ALU.add, accum_out=colA[:])
                sB = scr.tile([128, NM], F32, name="sB", tag="sB")
                nc.vector.tensor_tensor(out=sB[:], in0=M0sb[:], in1=astk[:],
                                        op=ALU.mult)
                nc.vector.tensor_reduce(q0c[:], sB[:],
                                        axis=mybir.AxisListType.X, op=ALU.add)

                # ---- pair-term exps (after the big exps in ACT order) ----
                nc.scalar.activation(distp[:], d2all[:], AF.Sqrt,
                                     bias=aux[:, 8:9])
                # pair exps; t_k via column-sum matmul then a fold matmul
                # into partitions 32k+p
                ps_t = psC.tile([75, 4], F32, name="ps_t", tag="sm", bufs=3)
                expks = []
                for k, kern in enumerate(KERNELS):
                    psrc = d2all if kern == "gaussian" else distp
                    expk = scr.tile([128, 78], F32, name=f"expk{k}",
                                    tag="expk", bufs=4)
                    nc.scalar.activation(expk[:], psrc[:], AF.Exp,
                                         scale=aux[:, 2 * k:2 * k + 1],
                                         bias=aux[:, 2 * k + 1:2 * k + 2])
                    expks.append(expk)
                    nc.tensor.matmul(ps_t[:, k:k + 1], expk[:, 0:75],
                                     ones[:, 0:1], start=True, stop=True)
                t75s = sml.tile([75, 4], F32, name="t75s")
                nc.vector.tensor_copy(t75s[:], ps_t[:])
                ps_tc = psC.tile([128, 1], F32, name="ps_tc", tag="sm", bufs=3)
                for k in range(4):
                    nc.tensor.matmul(ps_tc[32 * k:32 * k + 32, 0:1], fold[:],
                                     t75s[:, k:k + 1], start=True, stop=True,
                                     tile_position=(0, 32 * k),
                                     skip_group_check=True)
                tcol = sml.tile([128, 1], F32, name="tcol")
                nc.vector.tensor_scalar_mul(tcol[:], ps_tc[:], float(TCO))

                # ---- phase 6: corrections (col-tiled) and stripe sums ----
                ps_corr = psC.tile([128, 1], F32, name="ps_corr", tag="sm",
                                   bufs=3)
                for c in range(3):
                    for k in range(4):
                        nc.tensor.matmul(
                            ps_corr[32 * k:32 * k + 32, 0:1],
                            wct[:, 32 * c:32 * (c + 1)],
                            expks[k][:, 75 + c:76 + c],
                            start=(c == 0), stop=(c == 2),
                            tile_position=(0, 32 * k),
                            skip_group_check=True)
                sesum = sml.tile([3, 4], F32, name="sesum")
                for k in range(4):
                    ps_sek = psC.tile([3, 1], F32, name=f"ps_se{k}", tag="se",
                                      bufs=2)
                    nc.tensor.matmul(ps_sek[:], expks[k][:, 75:78],
                                     ones[:, 0:1], start=True, stop=True)
                    nc.vector.tensor_copy(sesum[:, k:k + 1], ps_sek[:])

                # ---- phase 7: U_b assembly in the stacked [128,1] layout ----
                colB = sml.tile([128, 1], F32, name="colB")
                nc.vector.tensor_tensor(out=colB[:], in0=arow[:], in1=colA[:],
                                        op=ALU.subtract)
                ubv = sml.tile([128, 1], F32, name="ubv")
                nc.vector.tensor_tensor(out=ubv[:], in0=q0c[:], in1=arow[:],
                                        op=ALU.subtract)
                nc.vector.tensor_scalar_mul(ubv[:], ubv[:], float(KAP))
                nc.vector.tensor_tensor(out=ubv[:], in0=ubv[:], in1=ps_corr[:],
                                        op=ALU.add)
                nc.vector.tensor_tensor(out=ubv[:], in0=ubv[:], in1=tcol[:],
                                        op=ALU.add)
                # transpose [ubv | colA | colB] at once -> rows
                stk = sml.tile([128, 3], F32, name="stk")
                nc.vector.tensor_copy(stk[:, 0:1], ubv[:])
                nc.vector.tensor_copy(stk[:, 1:2], colA[:])
                nc.vector.tensor_copy(stk[:, 2:3], colB[:])
                ps_stk = psC.tile([3, 128], F32, name="ps_stk", tag="sm",
                                  bufs=3)
                nc.tensor.transpose(ps_stk[:], stk[:], ident[:])
                srow = sml.tile([3, 128], F32, name="srow")
                nc.vector.tensor_copy(srow[:], ps_stk[:])

                # ---- phase 8: fold everything into one partition-0 row ----
                # frow: [0:128)=ub, [128:256)=colA^T, [256:384)=colB^T,
                # [384:396)=sesum
                frow = sml.tile([1, 396], F32, name="frow")
                nc.sync.dma_start(out=frow[0:1, 0:384], in_=srow[:])
                nc.sync.dma_start(out=frow[0:1, 384:396], in_=sesum[:])

                def fr(base, step=32, count=4):
                    ap = frow[0:1, base:base + 1]
                    return bass.AP(ap.tensor, ap.offset,
                                   [ap.ap[0], [step, count]])

                XXv = fr(128 + 25)
                YXv = fr(128 + 26)
                XY0v = fr(256 + 25)
                YYv = fr(256 + 26)
                # se_k = sum_c sesum[4c+k]
                sev = sml.tile([1, 4], F32, name="sev")
                nc.vector.tensor_reduce(
                    sev[:],
                    frow[0:1, 384:396].rearrange("o (c k) -> o k c", k=4),
                    axis=mybir.AxisListType.X, op=ALU.add)
                s0t = sml.tile([1, 4], F32, name="s0t")
                nc.vector.tensor_tensor(out=s0t[:], in0=XXv, in1=YXv,
                                        op=ALU.add)
                nc.vector.tensor_tensor(out=s0t[:], in0=s0t[:], in1=XY0v,
                                        op=ALU.add)
                nc.vector.tensor_tensor(out=s0t[:], in0=s0t[:], in1=YYv,
                                        op=ALU.add)
                ck = sml.tile([1, 4], F32, name="ck")
                nc.vector.tensor_tensor(out=ck[:], in0=s0t[:], in1=sev[:],
                                        op=ALU.subtract)
                nc.vector.tensor_tensor(out=ck[:], in0=ck[:],
                                        in1=aux4[0:1, 0:4], op=ALU.subtract)
                nc.vector.tensor_scalar_mul(ck[:], ck[:], float(IC1))
                u1 = sml.tile([1, 4], F32, name="u1")
                nc.vector.tensor_tensor(out=u1[:], in0=XXv, in1=YYv,
                                        op=ALU.add)
                nc.vector.tensor_tensor(out=u1[:], in0=u1[:],
                                        in1=aux4[0:1, 0:4], op=ALU.subtract)
                nc.vector.tensor_scalar_mul(u1[:], u1[:], float(IC1))
                u2 = sml.tile([1, 4], F32, name="u2")
                nc.vector.tensor_tensor(out=u2[:], in0=XY0v, in1=sev[:],
                                        op=ALU.subtract)
                nc.vector.tensor_scalar_mul(u2[:], u2[:], float(2.0 * IC2))

                # ---- phase 9: contiguous U row + U_b block, two out DMAs ----
                uF = sml.tile([1, 4], F32, name="uF")
                nc.vector.tensor_tensor(out=uF[:], in0=u1[:], in1=u2[:],
                                        op=ALU.subtract)
                ubc = sml.tile([1, 4 * PPC], F32, name="ubc")
                ub_src = frow[0:1, 0:128].rearrange("o (k p) -> o k p", p=32)
                ckap = ck[0:1, 0:4]
                ck_b = bass.AP(ckap.tensor, ckap.offset,
                               [ckap.ap[0], [1, 4], [0, PPC]])
                nc.vector.tensor_tensor(
                    out=ubc[0:1, :].rearrange("o (k p) -> o k p", p=PPC),
                    in0=ub_src[0:1, :, 0:PPC], in1=ck_b, op=ALU.add)
                nc.sync.dma_start(
                    out=out_d[:, 0:1],
                    in_=uF[0:1, :].rearrange("o (k w) -> o k w", w=1))
                nc.sync.dma_start(
                    out=out_d[:, 1:1 + PPC],
                    in_=ubc[0:1, :].rearrange("o (k p) -> o k p", p=PPC))

    nc.compile()
    return nc


def _host_prep(X, Y, bandwidths, perms):
    X = np.ascontiguousarray(X, np.float32)
    Y = np.ascontiguousarray(Y, np.float32)
    perms = np.ascontiguousarray(perms, np.int32)
    Zt = np.zeros((D + 1, NM), np.float32)  # rows 0-63 Zt, row 64 ones
    Zt[0:D] = np.concatenate([X, Y], 0).T
    Zt[D] = 1.0
    b = np.asarray(bandwidths, np.float64)
    gs = (-1.0 / (b * b)).astype(np.float32)
    gb = (gs.astype(np.float64) * 1e-12).astype(np.float32)
    ls = (-1.0 / b).astype(np.float32)
    aux = np.zeros((128, 10), np.float32)
    aux[:, 8] = 1e-12
    d0c = np.zeros(4, np.float64)
    for k, kern in enumerate(KERNELS):
        if kern == "gaussian":
            aux[:, 2 * k] = gs[k]
            aux[:, 2 * k + 1] = gb[k]
            d0c[k] = np.exp(-1e-12 / (b[k] * b[k]))
        else:
            aux[:, 2 * k] = ls[k]
            aux[:, 2 * k + 1] = 0.0
            d0c[k] = np.exp(-np.sqrt(1e-12) / b[k])
    aux4 = np.zeros((1, 8), np.float32)
    aux4[0, 0:4] = (768.0 * d0c).astype(np.float32)
    ident = np.eye(128, dtype=np.float32)
    foldm = np.zeros((75, 32), np.float32)
    foldm[:, :PPC] = (np.arange(75)[:, None] // 3 ==
                      np.arange(PPC)[None, :])

    maps = []
    for cid in range(NC):
        pm = perms[cid * PPC:(cid + 1) * PPC]
        A = np.zeros((27, NM), np.float32)
        A[np.arange(PPC)[:, None], pm[:, :N]] = 1
        A[25, :N] = 1
        A[26, N:] = 1
        astk = np.zeros((128, NM), np.float32)
        for k in range(4):
            astk[32 * k:32 * k + 27] = A
        atp = np.zeros((128, 6 * 32), np.float32)
        for c in range(6):
            atp[:, 32 * c:32 * c + 27] = A[:, 128 * c:128 * (c + 1)].T
        A1 = A[:PPC, :N]
        A2 = A[:PPC, N:]
        Wc = (-KAP * (A1 * A2) + CB1 * A1 + CB2 * A2).astype(np.float32)
        wct = np.zeros((128, 3 * 32), np.float32)
        for c in range(3):
            wct[:, 32 * c:32 * c + PPC] = Wc[:, 128 * c:128 * (c + 1)].T
        pX = pm[:, :N].astype(np.int64).ravel()
        pY = pm[:, N:].astype(np.int64).ravel()
        # Pair-arranged Z rows: [zx | zy] per pair; stripe pairs (pY==pX+384)
        # get a sentinel row with huge distance so f_k -> 0 (matches the
        # zeroed K stripe). Rows 9600..9983 are the stripe-diagonal pairs
        # (they produce the e_k correction vectors).
        Zf = np.concatenate([X, Y], 0)
        zx = Zf[pX]
        zy = Zf[pY]
        stripe = pY == pX + N
        zx[stripe] = 0.0
        zy[stripe] = 0.0
        zx[stripe, 0] = 1e6  # d2=1e12: exp(-1e12/b^2)=exp(-1e6/b)=0
        j = np.arange(N)
        zp = np.concatenate([
            np.concatenate([zx, zy], 1),
            np.concatenate([Zf[j], Zf[N + j]], 1),
        ], 0)
        import ml_dtypes
        zp = zp.reshape(78, 128, 128).transpose(1, 0, 2).reshape(128, 9984)
        zp = np.concatenate([zp, atp], 1).astype(ml_dtypes.bfloat16)
        bigin = np.zeros((128, 2002), np.float32)
        bigin[0:D + 1, 0:NM] = Zt
        bigin[:, NM:2 * NM] = astk
        bigin[:, 1536:1536 + 192] = atp
        bigin[:, 1728:1728 + 96] = wct
        bigin[0:75, 1824:1824 + 32] = foldm
        bigin[:, 1856:1856 + 128] = ident
        bigin[:, 1984:1994] = aux
        bigin[0:1, 1994:2002] = aux4
        maps.append(dict(bigin=bigin, zp=zp))
    return maps


_NC_CACHE = None


def _get_nc():
    global _NC_CACHE
    if _NC_CACHE is None:
        _NC_CACHE = _build()
    return _NC_CACHE


def kernel(X, Y, bandwidths, perms):
    nc = _get_nc()
    in_maps = _host_prep(X, Y, bandwidths, perms)
    res = bass_utils.run_bass_kernel_spmd(nc, in_maps, list(range(NC)))
    full = np.zeros((4, 1 + NPER), np.float32)
    full[:, 0] = res.results[0]["out"][:, 0]
    for cid in range(NC):
        full[:, 1 + cid * PPC:1 + (cid + 1) * PPC] = \
            res.results[cid]["out"][:, 1:]
    return full


# revision 33
# speedup vs baseline: 1.3134x; 1.0344x over previous
"""Trainium2 Bass kernel for the 4-kernel MMD permutation test (nn_DUAL_78237124264373).

Math (per core, 25 of the 200 permutations; everything else replicated):
  Z = [X; Y] (768 x 64), d2[r,c] = ||Z_r - Z_c||^2 built on the PE as a single
  rank-66 matmul  d2 = L^T R  with L = [Zt; sq; 1], R = [-2 Zt; 1; sq].
  K0_k = f_k(d2) (symmetric kernel matrix, no diag zeroing).
  With a_p the X-half indicator of permutation p and the zeroed-K statistics
  expressed through symmetric-K0 quantities plus corrections through
  e_j = K0[j, 384+j] (the zeroed stripe), every U_b entry reduces to
     U_b = kap*(q0 - arow0) + W_corr @ e_k + (2/c2)*t + C_k
  where q0 = a K0 a, arow0 = a K0 1 come from one matmul M0 = A_aug K0,
  t is the per-permutation paired-sample sum computed from host-arranged
  Z-row pairs (sentinel rows handle zeroed-stripe pairs), and W_corr folds
  the three correction coefficients into one host-built matrix.

Layout: the four kernels are column-tiled onto PE col-groups, so all
per-permutation statistics live at partition 32*k + p (kernel k, perm p) and
the DVE reductions run once over 128 partitions instead of 4x over 27.
"""

import os
import sys

import numpy as np

if "/opt/trn_rl_repo" not in sys.path:
    sys.path.insert(0, "/opt/trn_rl_repo")

import concourse.bacc as bacc
import concourse.bass as bass
import concourse.mybir as mybir
import concourse.tile as tile
from concourse import bass_utils

N = 384
NM = 768
D = 64
NPER = 200
NC = 8
PPC = NPER // NC  # 25
C1 = float(N * (N - 1))
C2 = float(N * N)
KAP = np.float32(2.0 / C1 + 2.0 / C2)
CB1 = np.float32(1.0 / C1 + 2.0 / C2)
CB2 = np.float32(1.0 / C1)
TCO = np.float32(2.0 / C2)
IC1 = np.float32(1.0 / C1)
IC2 = np.float32(1.0 / C2)
KERNELS = ("gaussian", "laplacian", "gaussian", "laplacian")

F32 = mybir.dt.float32
F32R = mybir.dt.float32r
BF16 = mybir.dt.bfloat16
AF = mybir.ActivationFunctionType
ALU = mybir.AluOpType


def _build():
    nc = bacc.Bacc("TRN2", target_bir_lowering=False, debug=False)
    with tile.TileContext(nc) as tc:
        with tc.tile_pool(name="dram", bufs=1, space="DRAM") as dram, \
             tc.tile_pool(name="io", bufs=1) as io, \
             tc.tile_pool(name="big", bufs=1) as big, \
             tc.tile_pool(name="kpool", bufs=4) as kpool, \
             tc.tile_pool(name="scr", bufs=2) as scr, \
             tc.tile_pool(name="sml", bufs=1) as sml:

            def din(name, shape, dt=F32):
                return dram.tile(shape, dt, kind="ExternalInput", name=name,
                                 uniquify=False)

            # One fused input tensor (single DMA); column layout below.
            W_IN = 2002
            bigin_d = din("bigin", [128, W_IN])
            zp_d = din("zp", [128, 9984 + 192], BF16)  # pair rows + bf16 atp
            out_d = dram.tile([4, 1 + PPC], F32, kind="ExternalOutput",
                              name="out", uniquify=False)

            # ---- phase 0: input DMAs (Zt block first: it gates the PE) ----
            bigin = io.tile([128, W_IN], F32, name="bigin_sb")
            nc.sync.dma_start(out=bigin[:, 0:NM], in_=bigin_d[:, 0:NM])
            nc.sync.dma_start(out=bigin[:, NM:], in_=bigin_d[:, NM:])
            Lbig = bigin[0:D + 1, 0:NM]          # [Zt rows 0-63; ones row 64]
            astk = bigin[:, NM:2 * NM]           # A_aug rows at 32k+i
            atp = bigin[:, 1536:1536 + 192]      # A_aug^T chunks (32-padded)
            wct = bigin[:, 1728:1728 + 96]       # W_corr^T chunks (32-padded)
            fold = bigin[0:75, 1824:1824 + 32]   # 3->1 fold (32-padded)
            ident = bigin[:, 1856:1856 + 128]
            aux = bigin[:, 1984:1994]
            aux4 = bigin[0:1, 1994:2002]
            zpf = io.tile([128, 78 * 128 + 192], BF16, name="zp_sb")
            nc.sync.dma_start(out=zpf[:], in_=zp_d[:])
            zp = zpf[:, 0:9984].rearrange("p (b d) -> p b d", d=128)
            atpb = zpf[:, 9984:9984 + 192]       # A_aug^T chunks in bf16

            ones = io.tile([128, 1], F32, name="ones_sb")
            nc.vector.memset(ones[:], 1.0)

            R_all = io.tile([D + 1, NM], F32, name="R_all")
            d2sb = big.tile([128, 6 * NM], F32, name="d2sb")
            dist = big.tile([128, 6 * NM], F32, name="dist_sb")
            M0sb = big.tile([128, NM], F32, name="M0sb")

            with tc.tile_pool(name="psA", bufs=2, space="PSUM") as psA:
                # ---- phase 1: sq = rowsums of Zt^2, landed at psum
                # partitions 0 (for the sq_col transposes) and 64 (for the
                # R matrix row) via col-tiling ----
                zt2 = scr.tile([D, NM], F32, name="zt2", tag="zt2", bufs=1)
                nc.vector.tensor_tensor(out=zt2[:], in0=Lbig[0:D, :],
                                        in1=Lbig[0:D, :], op=ALU.mult)
                ps_sq = psA.tile([128, NM], F32, name="ps_sq", tag="d2")
                for s in (slice(0, 512), slice(512, NM)):
                    nc.tensor.matmul(ps_sq[0:1, s], ones[0:D, 0:1], zt2[:, s],
                                     start=True, stop=True,
                                     skip_group_check=True)
                    nc.tensor.matmul(ps_sq[D:D + 1, s], ones[0:D, 0:1],
                                     zt2[:, s], start=True, stop=True,
                                     tile_position=(0, D),
                                     skip_group_check=True)
                sqrow = sml.tile([1, NM], F32, name="sqrow")
                nc.vector.tensor_copy(sqrow[:], ps_sq[0:1, :])
                # R = [-2 Zt; sq]; row 64 copies within partition 64.
                nc.vector.tensor_scalar_mul(R_all[0:D, :], Lbig[0:D, :], -2.0)
                nc.vector.tensor_copy(R_all[D:D + 1, :], ps_sq[D:D + 1, :])
                # sq as columns (for the relu bias): 6 tiny PE transposes
                ps_sqc = psA.tile([128, 8], F32, name="ps_sqc", tag="sqc",
                                  bufs=1)
                for r in range(6):
                    nc.tensor.matmul(ps_sqc[:, r:r + 1],
                                     sqrow[0:1, 128 * r:128 * (r + 1)],
                                     ones[0:1, 0:1], is_transpose=True,
                                     start=True, stop=True,
                                     skip_group_check=True)
                sqc = sml.tile([128, 8], F32, name="sqc")
                nc.vector.tensor_copy(sqc[:, 0:6], ps_sqc[:, 0:6])

                # ---- phase 2: psum = -2 Z Z^T + sq[c]; then
                # d2 = max(psum + sq[r], 0) on the DVE ----
                for r in range(6):
                    ps_d2 = psA.tile([128, NM], F32, name=f"ps_d2_{r}",
                                     tag="d2")
                    lhs = Lbig[:, 128 * r:128 * (r + 1)]
                    nc.tensor.matmul(ps_d2[:, 0:512], lhs, R_all[:, 0:512],
                                     start=True, stop=True)
                    nc.tensor.matmul(ps_d2[:, 512:NM], lhs, R_all[:, 512:NM],
                                     start=True, stop=True)
                    sl = slice(NM * r, NM * (r + 1))
                    nc.scalar.activation(d2sb[:, sl], ps_d2[:], AF.Relu,
                                         bias=sqc[:, r:r + 1])

            # ---- phase 3: pair distances from host-arranged Z-row pairs ----
            # zp rows j<9600: (Z[pX], Z[pY]) for pair j=(p*384+i); rows
            # 9600..9983 are the stripe pairs (Z[j], Z[384+j]) -> e_k.
            # 9600 = 75*128, so t-pairs land in cols 0..74 of d2all and the
            # stripe pairs in cols 75..77; perm p owns psum partitions 3p+c.
            pdiff = sml.tile([128, 78, 64], BF16, name="pdiff")
            nc.vector.tensor_tensor(out=pdiff[:], in0=zp[:, :, 0:64],
                                    in1=zp[:, :, 64:128], op=ALU.subtract)
            pprod = sml.tile([128, 78, 64], BF16, name="pprod")
            nc.vector.tensor_tensor(out=pprod[:], in0=pdiff[:], in1=pdiff[:],
                                    op=ALU.mult)
            d2all = sml.tile([128, 78], F32, name="d2all")
            nc.vector.tensor_reduce(d2all[:], pprod[:],
                                    axis=mybir.AxisListType.X, op=ALU.add)
            distp = sml.tile([128, 78], F32, name="distp")

            # ---- phase 4: dist = sqrt(d2 + 1e-12) (one wide ACT) ----
            nc.scalar.activation(dist[:], d2sb[:], AF.Sqrt, bias=aux[:, 8:9])

            arow = sml.tile([128, 1], F32, name="arow")
            colA = sml.tile([128, 1], F32, name="colA")
            q0c = sml.tile([128, 1], F32, name="q0c")

            with tc.tile_pool(name="psB", bufs=1, space="PSUM") as psB, \
                 tc.tile_pool(name="psC", bufs=1, space="PSUM") as psC:
                # ---- phase 5: K0_k = f_k(d2); M0 = A_aug K0 col-tiled so
                # kernel k's rows land at partitions 32k+i ----
                kts = []
                for k, kern in enumerate(KERNELS):
                    src = d2sb if kern == "gaussian" else dist
                    kt = kpool.tile([128, 6 * NM], BF16, name=f"kt{k}",
                                    tag="kt")
                    nc.scalar.activation(kt[:], src[:], AF.Exp,
                                         scale=aux[:, 2 * k:2 * k + 1],
                                         bias=aux[:, 2 * k + 1:2 * k + 2])
                    kts.append(kt)
                ps_m = psB.tile([128, NM], F32, name="ps_m")
                for c in range(6):
                    lhs = atpb[:, 32 * c:32 * (c + 1)]
                    for k in range(4):
                        pr = slice(32 * k, 32 * k + 32)
                        nc.tensor.matmul(ps_m[pr, 0:512], lhs,
                                         kts[k][:, NM * c:NM * c + 512],
                                         start=(c == 0), stop=(c == 5),
                                         tile_position=(0, 32 * k),
                                         skip_group_check=True)
                        nc.tensor.matmul(ps_m[pr, 512:NM], lhs,
                                         kts[k][:, NM * c + 512:NM * (c + 1)],
                                         start=(c == 0), stop=(c == 5),
                                         tile_position=(0, 32 * k),
                                         skip_group_check=True)
                # row stats: copy+rowsum fused, first-half sum, masked q0
                nc.vector.tensor_scalar(
                    out=M0sb[:], in0=ps_m[:], scalar1=1.0, scalar2=0.0,
                    op0=ALU.mult, op1=ALU.add, accum_out=arow[:])
                sA = scr.tile([128, N], F32, name="sA", tag="sA")
                nc.vector.tensor_scalar(
                    out=sA[:], in0=M0sb[:, 0:N], scalar1=1.0, scalar2=0.0,
                    op0=ALU.mult, op1=ALU.add, accum_out=colA[:])
                sB = scr.tile([128, NM], F32, name="sB", tag="sB")
                nc.vector.tensor_tensor(out=sB[:], in0=M0sb[:], in1=astk[:],
                                        op=ALU.mult)
                nc.vector.tensor_reduce(q0c[:], sB[:],
                                        axis=mybir.AxisListType.X, op=ALU.add)

                # ---- pair-term exps (after the big exps in ACT order) ----
                nc.scalar.activation(distp[:], d2all[:], AF.Sqrt,
                                     bias=aux[:, 8:9])
                # pair exps; t_k via column-sum matmul then a fold matmul
                # into partitions 32k+p
                ps_t = psC.tile([75, 4], F32, name="ps_t", tag="sm", bufs=3)
                expks = []
                for k, kern in enumerate(KERNELS):
                    psrc = d2all if kern == "gaussian" else distp
                    expk = scr.tile([128, 78], F32, name=f"expk{k}",
                                    tag="expk", bufs=4)
                    nc.scalar.activation(expk[:], psrc[:], AF.Exp,
                                         scale=aux[:, 2 * k:2 * k + 1],
                                         bias=aux[:, 2 * k + 1:2 * k + 2])
                    expks.append(expk)
                    nc.tensor.matmul(ps_t[:, k:k + 1], expk[:, 0:75],
                                     ones[:, 0:1], start=True, stop=True)
                t75s = sml.tile([75, 4], F32, name="t75s")
                nc.vector.tensor_copy(t75s[:], ps_t[:])
                ps_tc = psC.tile([128, 1], F32, name="ps_tc", tag="sm", bufs=3)
                for k in range(4):
                    nc.tensor.matmul(ps_tc[32 * k:32 * k + 32, 0:1], fold[:],
                                     t75s[:, k:k + 1], start=True, stop=True,
                                     tile_position=(0, 32 * k),
                                     skip_group_check=True)
                tcol = sml.tile([128, 1], F32, name="tcol")
                nc.vector.tensor_scalar_mul(tcol[:], ps_tc[:], float(TCO))

                # ---- phase 6: corrections (col-tiled) and stripe sums ----
                ps_corr = psC.tile([128, 1], F32, name="ps_corr", tag="sm",
                                   bufs=3)
                for c in range(3):
                    for k in range(4):
                        nc.tensor.matmul(
                            ps_corr[32 * k:32 * k + 32, 0:1],
                            wct[:, 32 * c:32 * (c + 1)],
                            expks[k][:, 75 + c:76 + c],
                            start=(c == 0), stop=(c == 2),
                            tile_position=(0, 32 * k),
                            skip_group_check=True)
                sesum = sml.tile([3, 4], F32, name="sesum")
                for k in range(4):
                    ps_sek = psC.tile([3, 1], F32, name=f"ps_se{k}", tag="se",
                                      bufs=2)
                    nc.tensor.matmul(ps_sek[:], expks[k][:, 75:78],
                                     ones[:, 0:1], start=True, stop=True)
                    nc.vector.tensor_copy(sesum[:, k:k + 1], ps_sek[:])

                # ---- phase 7: U_b assembly in the stacked [128,1] layout ----
                colB = sml.tile([128, 1], F32, name="colB")
                nc.vector.tensor_tensor(out=colB[:], in0=arow[:], in1=colA[:],
                                        op=ALU.subtract)
                ubv = sml.tile([128, 1], F32, name="ubv")
                nc.vector.tensor_tensor(out=ubv[:], in0=q0c[:], in1=arow[:],
                                        op=ALU.subtract)
                nc.vector.tensor_scalar_mul(ubv[:], ubv[:], float(KAP))
                nc.vector.tensor_tensor(out=ubv[:], in0=ubv[:], in1=ps_corr[:],
                                        op=ALU.add)
                nc.vector.tensor_tensor(out=ubv[:], in0=ubv[:], in1=tcol[:],
                                        op=ALU.add)
                # transpose [ubv | colA | colB] at once -> rows
                stk = sml.tile([128, 3], F32, name="stk")
                nc.vector.tensor_copy(stk[:, 0:1], ubv[:])
                nc.vector.tensor_copy(stk[:, 1:2], colA[:])
                nc.vector.tensor_copy(stk[:, 2:3], colB[:])
                ps_stk = psC.tile([3, 128], F32, name="ps_stk", tag="sm",
                                  bufs=3)
                nc.tensor.transpose(ps_stk[:], stk[:], ident[:])
                srow = sml.tile([3, 128], F32, name="srow")
                nc.vector.tensor_copy(srow[:], ps_stk[:])

                # ---- phase 8: fold everything into one partition-0 row ----
                # frow: [0:128)=ub, [128:256)=colA^T, [256:384)=colB^T,
                # [384:396)=sesum
                frow = sml.tile([1, 396], F32, name="frow")
                nc.sync.dma_start(out=frow[0:1, 0:384], in_=srow[:])
                nc.sync.dma_start(out=frow[0:1, 384:396], in_=sesum[:])

                def fr(base, step=32, count=4):
                    ap = frow[0:1, base:base + 1]
                    return bass.AP(ap.tensor, ap.offset,
                                   [ap.ap[0], [step, count]])

                XXv = fr(128 + 25)
                YXv = fr(128 + 26)
                XY0v = fr(256 + 25)
                YYv = fr(256 + 26)
                # se_k = sum_c sesum[4c+k]
                sev = sml.tile([1, 4], F32, name="sev")
                nc.vector.tensor_reduce(
                    sev[:],
                    frow[0:1, 384:396].rearrange("o (c k) -> o k c", k=4),
                    axis=mybir.AxisListType.X, op=ALU.add)
                s0t = sml.tile([1, 4], F32, name="s0t")
                nc.vector.tensor_tensor(out=s0t[:], in0=XXv, in1=YXv,
                                        op=ALU.add)
                nc.vector.tensor_tensor(out=s0t[:], in0=s0t[:], in1=XY0v,
                                        op=ALU.add)
                nc.vector.tensor_tensor(out=s0t[:], in0=s0t[:], in1=YYv,
                                        op=ALU.add)
                ck = sml.tile([1, 4], F32, name="ck")
                nc.vector.tensor_tensor(out=ck[:], in0=s0t[:], in1=sev[:],
                                        op=ALU.subtract)
                nc.vector.tensor_tensor(out=ck[:], in0=ck[:],
                                        in1=aux4[0:1, 0:4], op=ALU.subtract)
                nc.vector.tensor_scalar_mul(ck[:], ck[:], float(IC1))
                u1 = sml.tile([1, 4], F32, name="u1")
                nc.vector.tensor_tensor(out=u1[:], in0=XXv, in1=YYv,
                                        op=ALU.add)
                nc.vector.tensor_tensor(out=u1[:], in0=u1[:],
                                        in1=aux4[0:1, 0:4], op=ALU.subtract)
                nc.vector.tensor_scalar_mul(u1[:], u1[:], float(IC1))
                u2 = sml.tile([1, 4], F32, name="u2")
                nc.vector.tensor_tensor(out=u2[:], in0=XY0v, in1=sev[:],
                                        op=ALU.subtract)
                nc.vector.tensor_scalar_mul(u2[:], u2[:], float(2.0 * IC2))

                # ---- phase 9: contiguous U row + U_b block, two out DMAs ----
                uF = sml.tile([1, 4], F32, name="uF")
                nc.vector.tensor_tensor(out=uF[:], in0=u1[:], in1=u2[:],
                                        op=ALU.subtract)
                ubc = sml.tile([1, 4 * PPC], F32, name="ubc")
                ub_src = frow[0:1, 0:128].rearrange("o (k p) -> o k p", p=32)
                ckap = ck[0:1, 0:4]
                ck_b = bass.AP(ckap.tensor, ckap.offset,
                               [ckap.ap[0], [1, 4], [0, PPC]])
                nc.vector.tensor_tensor(
                    out=ubc[0:1, :].rearrange("o (k p) -> o k p", p=PPC),
                    in0=ub_src[0:1, :, 0:PPC], in1=ck_b, op=ALU.add)
                nc.sync.dma_start(
                    out=out_d[:, 0:1],
                    in_=uF[0:1, :].rearrange("o (k w) -> o k w", w=1))
                nc.sync.dma_start(
                    out=out_d[:, 1:1 + PPC],
                    in_=ubc[0:1, :].rearrange("o (k p) -> o k p", p=PPC))

    nc.compile()
    return nc


def _host_prep(X, Y, bandwidths, perms):
    X = np.ascontiguousarray(X, np.float32)
    Y = np.ascontiguousarray(Y, np.float32)
    perms = np.ascontiguousarray(perms, np.int32)
    Zt = np.zeros((D + 1, NM), np.float32)  # rows 0-63 Zt, row 64 ones
    Zt[0:D] = np.concatenate([X, Y], 0).T
    Zt[D] = 1.0
    b = np.asarray(bandwidths, np.float64)
    gs = (-1.0 / (b * b)).astype(np.float32)
    gb = (gs.astype(np.float64) * 1e-12).astype(np.float32)
    ls = (-1.0 / b).astype(np.float32)
    aux = np.zeros((128, 10), np.float32)
    aux[:, 8] = 1e-12
    d0c = np.zeros(4, np.float64)
    for k, kern in enumerate(KERNELS):
        if kern == "gaussian":
            aux[:, 2 * k] = gs[k]
            aux[:, 2 * k + 1] = gb[k]
            d0c[k] = np.exp(-1e-12 / (b[k] * b[k]))
        else:
            aux[:, 2 * k] = ls[k]
            aux[:, 2 * k + 1] = 0.0
            d0c[k] = np.exp(-np.sqrt(1e-12) / b[k])
    aux4 = np.zeros((1, 8), np.float32)
    aux4[0, 0:4] = (768.0 * d0c).astype(np.float32)
    ident = np.eye(128, dtype=np.float32)
    foldm = np.zeros((75, 32), np.float32)
    foldm[:, :PPC] = (np.arange(75)[:, None] // 3 ==
                      np.arange(PPC)[None, :])

    maps = []
    for cid in range(NC):
        pm = perms[cid * PPC:(cid + 1) * PPC]
        A = np.zeros((27, NM), np.float32)
        A[np.arange(PPC)[:, None], pm[:, :N]] = 1
        A[25, :N] = 1
        A[26, N:] = 1
        astk = np.zeros((128, NM), np.float32)
        for k in range(4):
            astk[32 * k:32 * k + 27] = A
        atp = np.zeros((128, 6 * 32), np.float32)
        for c in range(6):
            atp[:, 32 * c:32 * c + 27] = A[:, 128 * c:128 * (c + 1)].T
        A1 = A[:PPC, :N]
        A2 = A[:PPC, N:]
        Wc = (-KAP * (A1 * A2) + CB1 * A1 + CB2 * A2).astype(np.float32)
        wct = np.zeros((128, 3 * 32), np.float32)
        for c in range(3):
            wct[:, 32 * c:32 * c + PPC] = Wc[:, 128 * c:128 * (c + 1)].T
        pX = pm[:, :N].astype(np.int64).ravel()
        pY = pm[:, N:].astype(np.int64).ravel()
        # Pair-arranged Z rows: [zx | zy] per pair; stripe pairs (pY==pX+384)
        # get a sentinel row with huge distance so f_k -> 0 (matches the
        # zeroed K stripe). Rows 9600..9983 are the stripe-diagonal pairs
        # (they produce the e_k correction vectors).
        Zf = np.concatenate([X, Y], 0)
        zx = Zf[pX]
        zy = Zf[pY]
        stripe = pY == pX + N
        zx[stripe] = 0.0
        zy[stripe] = 0.0
        zx[stripe, 0] = 1e6  # d2=1e12: exp(-1e12/b^2)=exp(-1e6/b)=0
        j = np.arange(N)
        zp = np.concatenate([
            np.concatenate([zx, zy], 1),
            np.concatenate([Zf[j], Zf[N + j]], 1),
        ], 0)
        import ml_dtypes
        zp = zp.reshape(78, 128, 128).transpose(1, 0, 2).reshape(128, 9984)
        zp = np.concatenate([zp, atp], 1).astype(ml_dtypes.bfloat16)
        bigin = np.zeros((128, 2002), np.float32)
        bigin[0:D + 1, 0:NM] = Zt
        bigin[:, NM:2 * NM] = astk
        bigin[:, 1536:1536 + 192] = atp
        bigin[:, 1728:1728 + 96] = wct
        bigin[0:75, 1824:1824 + 32] = foldm
        bigin[:, 1856:1856 + 128] = ident
        bigin[:, 1984:1994] = aux
        bigin[0:1, 1994:2002] = aux4
        maps.append(dict(bigin=bigin, zp=zp))
    return maps


_NC_CACHE = None


def _get_nc():
    global _NC_CACHE
    if _NC_CACHE is None:
        _NC_CACHE = _build()
    return _NC_CACHE


def kernel(X, Y, bandwidths, perms):
    nc = _get_nc()
    in_maps = _host_prep(X, Y, bandwidths, perms)
    res = bass_utils.run_bass_kernel_spmd(nc, in_maps, list(range(NC)))
    full = np.zeros((4, 1 + NPER), np.float32)
    full[:, 0] = res.results[0]["out"][:, 0]
    for cid in range(NC):
        full[:, 1 + cid * PPC:1 + (cid + 1) * PPC] = \
            res.results[cid]["out"][:, 1:]
    return full


# revision 34
# speedup vs baseline: 1.3712x; 1.0440x over previous
"""Trainium2 Bass kernel for the 4-kernel MMD permutation test (nn_DUAL_78237124264373).

Math (per core, 25 of the 200 permutations; everything else replicated):
  Z = [X; Y] (768 x 64), d2[r,c] = ||Z_r - Z_c||^2 built on the PE as a single
  rank-66 matmul  d2 = L^T R  with L = [Zt; sq; 1], R = [-2 Zt; 1; sq].
  K0_k = f_k(d2) (symmetric kernel matrix, no diag zeroing).
  With a_p the X-half indicator of permutation p and the zeroed-K statistics
  expressed through symmetric-K0 quantities plus corrections through
  e_j = K0[j, 384+j] (the zeroed stripe), every U_b entry reduces to
     U_b = kap*(q0 - arow0) + W_corr @ e_k + (2/c2)*t + C_k
  where q0 = a K0 a, arow0 = a K0 1 come from one matmul M0 = A_aug K0,
  t is the per-permutation paired-sample sum computed from host-arranged
  Z-row pairs (sentinel rows handle zeroed-stripe pairs), and W_corr folds
  the three correction coefficients into one host-built matrix.

Layout: the four kernels are column-tiled onto PE col-groups, so all
per-permutation statistics live at partition 32*k + p (kernel k, perm p) and
the DVE reductions run once over 128 partitions instead of 4x over 27.
"""

import os
import sys

import numpy as np

if "/opt/trn_rl_repo" not in sys.path:
    sys.path.insert(0, "/opt/trn_rl_repo")

import concourse.bacc as bacc
import concourse.bass as bass
import concourse.mybir as mybir
import concourse.tile as tile
from concourse import bass_utils

N = 384
NM = 768
D = 64
NPER = 200
NC = 8
PPC = NPER // NC  # 25
C1 = float(N * (N - 1))
C2 = float(N * N)
KAP = np.float32(2.0 / C1 + 2.0 / C2)
CB1 = np.float32(1.0 / C1 + 2.0 / C2)
CB2 = np.float32(1.0 / C1)
TCO = np.float32(2.0 / C2)
IC1 = np.float32(1.0 / C1)
IC2 = np.float32(1.0 / C2)
KERNELS = ("gaussian", "laplacian", "gaussian", "laplacian")

F32 = mybir.dt.float32
F32R = mybir.dt.float32r
BF16 = mybir.dt.bfloat16
AF = mybir.ActivationFunctionType
ALU = mybir.AluOpType


def _build():
    nc = bacc.Bacc("TRN2", target_bir_lowering=False, debug=False)
    with tile.TileContext(nc) as tc:
        with tc.tile_pool(name="dram", bufs=1, space="DRAM") as dram, \
             tc.tile_pool(name="io", bufs=1) as io, \
             tc.tile_pool(name="big", bufs=1) as big, \
             tc.tile_pool(name="kpool", bufs=4) as kpool, \
             tc.tile_pool(name="scr", bufs=2) as scr, \
             tc.tile_pool(name="sml", bufs=1) as sml:

            def din(name, shape, dt=F32):
                return dram.tile(shape, dt, kind="ExternalInput", name=name,
                                 uniquify=False)

            # One fused input tensor (single DMA); column layout below.
            W_IN = 2002
            bigin_d = din("bigin", [128, W_IN])
            zp_d = din("zp", [128, 9984 + 192], BF16)  # pair rows + bf16 atp
            out_d = dram.tile([4, 1 + PPC], F32, kind="ExternalOutput",
                              name="out", uniquify=False)

            # ---- phase 0: input DMAs (Zt block first: it gates the PE) ----
            bigin = io.tile([128, W_IN], F32, name="bigin_sb")
            nc.sync.dma_start(out=bigin[:, 0:NM], in_=bigin_d[:, 0:NM])
            nc.sync.dma_start(out=bigin[:, NM:], in_=bigin_d[:, NM:])
            Lbig = bigin[0:D + 1, 0:NM]          # [Zt rows 0-63; ones row 64]
            astk = bigin[:, NM:2 * NM]           # A_aug rows at 32k+i
            atp = bigin[:, 1536:1536 + 192]      # A_aug^T chunks (32-padded)
            wct = bigin[:, 1728:1728 + 96]       # W_corr^T chunks (32-padded)
            fold = bigin[0:75, 1824:1824 + 32]   # 3->1 fold (32-padded)
            ident = bigin[:, 1856:1856 + 128]
            aux = bigin[:, 1984:1994]
            aux4 = bigin[0:1, 1994:2002]
            zpf = io.tile([128, 78 * 128 + 192], BF16, name="zp_sb")
            nc.sync.dma_start(out=zpf[:], in_=zp_d[:])
            zp = zpf[:, 0:9984].rearrange("p (b d) -> p b d", d=128)
            atpb = zpf[:, 9984:9984 + 192]       # A_aug^T chunks in bf16

            ones = io.tile([128, 1], F32, name="ones_sb")
            nc.vector.memset(ones[:], 1.0)

            R_all = io.tile([D + 1, NM], F32, name="R_all")
            d2sb = big.tile([128, 6 * NM], F32, name="d2sb")
            dist = big.tile([128, 6 * NM], F32, name="dist_sb")
            M0sb = big.tile([128, NM], F32, name="M0sb")

            with tc.tile_pool(name="psA", bufs=3, space="PSUM") as psA:
                # ---- phase 1: sq = rowsums of Zt^2, landed at psum
                # partitions 0 (for the sq_col transposes) and 64 (for the
                # R matrix row) via col-tiling ----
                zt2 = scr.tile([D, NM], F32, name="zt2", tag="zt2", bufs=1)
                nc.vector.tensor_tensor(out=zt2[:], in0=Lbig[0:D, :],
                                        in1=Lbig[0:D, :], op=ALU.mult)
                ps_sq = psA.tile([128, NM], F32, name="ps_sq", tag="d2")
                for s in (slice(0, 512), slice(512, NM)):
                    nc.tensor.matmul(ps_sq[0:1, s], ones[0:D, 0:1], zt2[:, s],
                                     start=True, stop=True,
                                     skip_group_check=True)
                    nc.tensor.matmul(ps_sq[D:D + 1, s], ones[0:D, 0:1],
                                     zt2[:, s], start=True, stop=True,
                                     tile_position=(0, D),
                                     skip_group_check=True)
                sqrow = sml.tile([1, NM], F32, name="sqrow")
                nc.vector.tensor_copy(sqrow[:], ps_sq[0:1, :])
                # R = [-2 Zt; sq]; row 64 copies within partition 64.
                nc.vector.tensor_scalar_mul(R_all[0:D, :], Lbig[0:D, :], -2.0)
                nc.vector.tensor_copy(R_all[D:D + 1, :], ps_sq[D:D + 1, :])
                # sq as columns (for the relu bias): 6 tiny PE transposes
                ps_sqc = psA.tile([128, 8], F32, name="ps_sqc", tag="sqc",
                                  bufs=1)
                for r in range(6):
                    nc.tensor.matmul(ps_sqc[:, r:r + 1],
                                     sqrow[0:1, 128 * r:128 * (r + 1)],
                                     ones[0:1, 0:1], is_transpose=True,
                                     start=True, stop=True,
                                     skip_group_check=True)
                sqc = sml.tile([128, 8], F32, name="sqc")
                nc.vector.tensor_copy(sqc[:, 0:6], ps_sqc[:, 0:6])

                # ---- phase 2: psum = -2 Z Z^T + sq[c]; then
                # d2 = max(psum + sq[r], 0) on the DVE ----
                for r in range(6):
                    ps_d2 = psA.tile([128, NM], F32, name=f"ps_d2_{r}",
                                     tag="d2")
                    lhs = Lbig[:, 128 * r:128 * (r + 1)]
                    nc.tensor.matmul(ps_d2[:, 0:512], lhs, R_all[:, 0:512],
                                     start=True, stop=True)
                    nc.tensor.matmul(ps_d2[:, 512:NM], lhs, R_all[:, 512:NM],
                                     start=True, stop=True)
                    sl = slice(NM * r, NM * (r + 1))
                    nc.vector.tensor_scalar(
                        out=d2sb[:, sl], in0=ps_d2[:],
                        scalar1=sqc[:, r:r + 1], scalar2=0.0,
                        op0=ALU.add, op1=ALU.max)

            # ---- phase 3: pair distances from host-arranged Z-row pairs ----
            # zp rows j<9600: (Z[pX], Z[pY]) for pair j=(p*384+i); rows
            # 9600..9983 are the stripe pairs (Z[j], Z[384+j]) -> e_k.
            # 9600 = 75*128, so t-pairs land in cols 0..74 of d2all and the
            # stripe pairs in cols 75..77; perm p owns psum partitions 3p+c.
            pdiff = sml.tile([128, 78, 64], BF16, name="pdiff")
            nc.gpsimd.tensor_tensor(out=pdiff[:], in0=zp[:, :, 0:64],
                                    in1=zp[:, :, 64:128], op=ALU.subtract)
            pprod = sml.tile([128, 78, 64], BF16, name="pprod")
            nc.gpsimd.tensor_tensor(out=pprod[:], in0=pdiff[:], in1=pdiff[:],
                                    op=ALU.mult)
            d2all = sml.tile([128, 78], F32, name="d2all")
            nc.vector.tensor_reduce(d2all[:], pprod[:],
                                    axis=mybir.AxisListType.X, op=ALU.add)
            distp = sml.tile([128, 78], F32, name="distp")

            # ---- phase 4: dist = sqrt(d2 + 1e-12) (one wide ACT) ----
            nc.scalar.activation(dist[:], d2sb[:], AF.Sqrt, bias=aux[:, 8:9])

            arow = sml.tile([128, 1], F32, name="arow")
            colA = sml.tile([128, 1], F32, name="colA")
            q0c = sml.tile([128, 1], F32, name="q0c")

            with tc.tile_pool(name="psB", bufs=1, space="PSUM") as psB, \
                 tc.tile_pool(name="psC", bufs=1, space="PSUM") as psC:
                # ---- phase 5: K0_k = f_k(d2); M0 = A_aug K0 col-tiled so
                # kernel k's rows land at partitions 32k+i ----
                kts = []
                ps_m = psB.tile([128, NM], F32, name="ps_m")
                for k, kern in enumerate(KERNELS):
                    src = d2sb if kern == "gaussian" else dist
                    kt = kpool.tile([128, 6 * NM], BF16, name=f"kt{k}",
                                    tag="kt")
                    for h in range(2):
                        hs = slice(3 * NM * h, 3 * NM * (h + 1))
                        nc.scalar.activation(kt[:, hs], src[:, hs], AF.Exp,
                                             scale=aux[:, 2 * k:2 * k + 1],
                                             bias=aux[:, 2 * k + 1:2 * k + 2])
                    kts.append(kt)
                for c in range(6):
                    lhs = atpb[:, 32 * c:32 * (c + 1)]
                    for k in range(4):
                        pr = slice(32 * k, 32 * k + 32)
                        nc.tensor.matmul(ps_m[pr, 0:512], lhs,
                                         kts[k][:, NM * c:NM * c + 512],
                                         start=(c == 0), stop=(c == 5),
                                         tile_position=(0, 32 * k),
                                         skip_group_check=True)
                        nc.tensor.matmul(ps_m[pr, 512:NM], lhs,
                                         kts[k][:, NM * c + 512:NM * (c + 1)],
                                         start=(c == 0), stop=(c == 5),
                                         tile_position=(0, 32 * k),
                                         skip_group_check=True)
                # row stats: copy+rowsum fused, first-half sum, masked q0
                nc.vector.tensor_scalar(
                    out=M0sb[:], in0=ps_m[:], scalar1=1.0, scalar2=0.0,
                    op0=ALU.mult, op1=ALU.add, accum_out=arow[:])
                sA = scr.tile([128, N], F32, name="sA", tag="sA")
                nc.vector.tensor_scalar(
                    out=sA[:], in0=M0sb[:, 0:N], scalar1=1.0, scalar2=0.0,
                    op0=ALU.mult, op1=ALU.add, accum_out=colA[:])
                sB = scr.tile([128, NM], F32, name="sB", tag="sB")
                nc.vector.tensor_tensor(out=sB[:], in0=M0sb[:], in1=astk[:],
                                        op=ALU.mult)
                nc.vector.tensor_reduce(q0c[:], sB[:],
                                        axis=mybir.AxisListType.X, op=ALU.add)

                # ---- pair-term exps (after the big exps in ACT order) ----
                nc.scalar.activation(distp[:], d2all[:], AF.Sqrt,
                                     bias=aux[:, 8:9])
                # pair exps; t_k via column-sum matmul then a fold matmul
                # into partitions 32k+p
                ps_t = psC.tile([75, 4], F32, name="ps_t", tag="sm", bufs=3)
                expks = []
                for k, kern in enumerate(KERNELS):
                    psrc = d2all if kern == "gaussian" else distp
                    expk = scr.tile([128, 78], F32, name=f"expk{k}",
                                    tag="expk", bufs=4)
                    nc.scalar.activation(expk[:], psrc[:], AF.Exp,
                                         scale=aux[:, 2 * k:2 * k + 1],
                                         bias=aux[:, 2 * k + 1:2 * k + 2])
                    expks.append(expk)
                    nc.tensor.matmul(ps_t[:, k:k + 1], expk[:, 0:75],
                                     ones[:, 0:1], start=True, stop=True)
                t75s = sml.tile([75, 4], F32, name="t75s")
                nc.vector.tensor_copy(t75s[:], ps_t[:])
                ps_tc = psC.tile([128, 1], F32, name="ps_tc", tag="sm", bufs=3)
                for k in range(4):
                    nc.tensor.matmul(ps_tc[32 * k:32 * k + 32, 0:1], fold[:],
                                     t75s[:, k:k + 1], start=True, stop=True,
                                     tile_position=(0, 32 * k),
                                     skip_group_check=True)
                tcol = sml.tile([128, 1], F32, name="tcol")
                nc.vector.tensor_scalar_mul(tcol[:], ps_tc[:], float(TCO))

                # ---- phase 6: corrections (col-tiled) and stripe sums ----
                ps_corr = psC.tile([128, 1], F32, name="ps_corr", tag="sm",
                                   bufs=3)
                for c in range(3):
                    for k in range(4):
                        nc.tensor.matmul(
                            ps_corr[32 * k:32 * k + 32, 0:1],
                            wct[:, 32 * c:32 * (c + 1)],
                            expks[k][:, 75 + c:76 + c],
                            start=(c == 0), stop=(c == 2),
                            tile_position=(0, 32 * k),
                            skip_group_check=True)
                sesum = sml.tile([3, 4], F32, name="sesum")
                for k in range(4):
                    ps_sek = psC.tile([3, 1], F32, name=f"ps_se{k}", tag="se",
                                      bufs=2)
                    nc.tensor.matmul(ps_sek[:], expks[k][:, 75:78],
                                     ones[:, 0:1], start=True, stop=True)
                    nc.vector.tensor_copy(sesum[:, k:k + 1], ps_sek[:])

                # ---- phase 7: U_b assembly in the stacked [128,1] layout ----
                colB = sml.tile([128, 1], F32, name="colB")
                nc.vector.tensor_tensor(out=colB[:], in0=arow[:], in1=colA[:],
                                        op=ALU.subtract)
                ubv = sml.tile([128, 1], F32, name="ubv")
                nc.vector.tensor_tensor(out=ubv[:], in0=q0c[:], in1=arow[:],
                                        op=ALU.subtract)
                nc.vector.tensor_scalar_mul(ubv[:], ubv[:], float(KAP))
                nc.vector.tensor_tensor(out=ubv[:], in0=ubv[:], in1=ps_corr[:],
                                        op=ALU.add)
                nc.vector.tensor_tensor(out=ubv[:], in0=ubv[:], in1=tcol[:],
                                        op=ALU.add)
                # ---- phase 8: fold everything into one partition-0 row ----
                # frow: [0:128)=ub, [128:256)=colA^T, [256:384)=colB^T,
                # [384:396)=sesum
                frow = sml.tile([1, 396], F32, name="frow")
                nc.sync.dma_start(out=frow[0:1, 0:128], in_=ubv[:])
                nc.sync.dma_start(out=frow[0:1, 128:256], in_=colA[:])
                nc.sync.dma_start(out=frow[0:1, 256:384], in_=colB[:])
                nc.sync.dma_start(out=frow[0:1, 384:396], in_=sesum[:])

                def fr(base, step=32, count=4):
                    ap = frow[0:1, base:base + 1]
                    return bass.AP(ap.tensor, ap.offset,
                                   [ap.ap[0], [step, count]])

                XXv = fr(128 + 25)
                YXv = fr(128 + 26)
                XY0v = fr(256 + 25)
                YYv = fr(256 + 26)
                # se_k = sum_c sesum[4c+k]
                sev = sml.tile([1, 4], F32, name="sev")
                nc.vector.tensor_reduce(
                    sev[:],
                    frow[0:1, 384:396].rearrange("o (c k) -> o k c", k=4),
                    axis=mybir.AxisListType.X, op=ALU.add)
                s0t = sml.tile([1, 4], F32, name="s0t")
                nc.vector.tensor_tensor(out=s0t[:], in0=XXv, in1=YXv,
                                        op=ALU.add)
                nc.vector.tensor_tensor(out=s0t[:], in0=s0t[:], in1=XY0v,
                                        op=ALU.add)
                nc.vector.tensor_tensor(out=s0t[:], in0=s0t[:], in1=YYv,
                                        op=ALU.add)
                ck = sml.tile([1, 4], F32, name="ck")
                nc.vector.tensor_tensor(out=ck[:], in0=s0t[:], in1=sev[:],
                                        op=ALU.subtract)
                nc.vector.tensor_tensor(out=ck[:], in0=ck[:],
                                        in1=aux4[0:1, 0:4], op=ALU.subtract)
                nc.vector.tensor_scalar_mul(ck[:], ck[:], float(IC1))
                u1 = sml.tile([1, 4], F32, name="u1")
                nc.vector.tensor_tensor(out=u1[:], in0=XXv, in1=YYv,
                                        op=ALU.add)
                nc.vector.tensor_tensor(out=u1[:], in0=u1[:],
                                        in1=aux4[0:1, 0:4], op=ALU.subtract)
                nc.vector.tensor_scalar_mul(u1[:], u1[:], float(IC1))
                u2 = sml.tile([1, 4], F32, name="u2")
                nc.vector.tensor_tensor(out=u2[:], in0=XY0v, in1=sev[:],
                                        op=ALU.subtract)
                nc.vector.tensor_scalar_mul(u2[:], u2[:], float(2.0 * IC2))

                # ---- phase 9: contiguous U row + U_b block, two out DMAs ----
                uF = sml.tile([1, 4], F32, name="uF")
                nc.vector.tensor_tensor(out=uF[:], in0=u1[:], in1=u2[:],
                                        op=ALU.subtract)
                ubc = sml.tile([1, 4 * PPC], F32, name="ubc")
                ub_src = frow[0:1, 0:128].rearrange("o (k p) -> o k p", p=32)
                ckap = ck[0:1, 0:4]
                ck_b = bass.AP(ckap.tensor, ckap.offset,
                               [ckap.ap[0], [1, 4], [0, PPC]])
                nc.vector.tensor_tensor(
                    out=ubc[0:1, :].rearrange("o (k p) -> o k p", p=PPC),
                    in0=ub_src[0:1, :, 0:PPC], in1=ck_b, op=ALU.add)
                nc.sync.dma_start(
                    out=out_d[:, 0:1],
                    in_=uF[0:1, :].rearrange("o (k w) -> o k w", w=1))
                nc.sync.dma_start(
                    out=out_d[:, 1:1 + PPC],
                    in_=ubc[0:1, :].rearrange("o (k p) -> o k p", p=PPC))

    nc.compile()
    return nc


def _host_prep(X, Y, bandwidths, perms):
    X = np.ascontiguousarray(X, np.float32)
    Y = np.ascontiguousarray(Y, np.float32)
    perms = np.ascontiguousarray(perms, np.int32)
    Zt = np.zeros((D + 1, NM), np.float32)  # rows 0-63 Zt, row 64 ones
    Zt[0:D] = np.concatenate([X, Y], 0).T
    Zt[D] = 1.0
    b = np.asarray(bandwidths, np.float64)
    gs = (-1.0 / (b * b)).astype(np.float32)
    gb = (gs.astype(np.float64) * 1e-12).astype(np.float32)
    ls = (-1.0 / b).astype(np.float32)
    aux = np.zeros((128, 10), np.float32)
    aux[:, 8] = 1e-12
    d0c = np.zeros(4, np.float64)
    for k, kern in enumerate(KERNELS):
        if kern == "gaussian":
            aux[:, 2 * k] = gs[k]
            aux[:, 2 * k + 1] = gb[k]
            d0c[k] = np.exp(-1e-12 / (b[k] * b[k]))
        else:
            aux[:, 2 * k] = ls[k]
            aux[:, 2 * k + 1] = 0.0
            d0c[k] = np.exp(-np.sqrt(1e-12) / b[k])
    aux4 = np.zeros((1, 8), np.float32)
    aux4[0, 0:4] = (768.0 * d0c).astype(np.float32)
    ident = np.eye(128, dtype=np.float32)
    foldm = np.zeros((75, 32), np.float32)
    foldm[:, :PPC] = (np.arange(75)[:, None] // 3 ==
                      np.arange(PPC)[None, :])

    maps = []
    for cid in range(NC):
        pm = perms[cid * PPC:(cid + 1) * PPC]
        A = np.zeros((27, NM), np.float32)
        A[np.arange(PPC)[:, None], pm[:, :N]] = 1
        A[25, :N] = 1
        A[26, N:] = 1
        astk = np.zeros((128, NM), np.float32)
        for k in range(4):
            astk[32 * k:32 * k + 27] = A
        atp = np.zeros((128, 6 * 32), np.float32)
        for c in range(6):
            atp[:, 32 * c:32 * c + 27] = A[:, 128 * c:128 * (c + 1)].T
        A1 = A[:PPC, :N]
        A2 = A[:PPC, N:]
        Wc = (-KAP * (A1 * A2) + CB1 * A1 + CB2 * A2).astype(np.float32)
        wct = np.zeros((128, 3 * 32), np.float32)
        for c in range(3):
            wct[:, 32 * c:32 * c + PPC] = Wc[:, 128 * c:128 * (c + 1)].T
        pX = pm[:, :N].astype(np.int64).ravel()
        pY = pm[:, N:].astype(np.int64).ravel()
        # Pair-arranged Z rows: [zx | zy] per pair; stripe pairs (pY==pX+384)
        # get a sentinel row with huge distance so f_k -> 0 (matches the
        # zeroed K stripe). Rows 9600..9983 are the stripe-diagonal pairs
        # (they produce the e_k correction vectors).
        Zf = np.concatenate([X, Y], 0)
        zx = Zf[pX]
        zy = Zf[pY]
        stripe = pY == pX + N
        zx[stripe] = 0.0
        zy[stripe] = 0.0
        zx[stripe, 0] = 1e6  # d2=1e12: exp(-1e12/b^2)=exp(-1e6/b)=0
        j = np.arange(N)
        zp = np.concatenate([
            np.concatenate([zx, zy], 1),
            np.concatenate([Zf[j], Zf[N + j]], 1),
        ], 0)
        import ml_dtypes
        zp = zp.reshape(78, 128, 128).transpose(1, 0, 2).reshape(128, 9984)
        zp = np.concatenate([zp, atp], 1).astype(ml_dtypes.bfloat16)
        bigin = np.zeros((128, 2002), np.float32)
        bigin[0:D + 1, 0:NM] = Zt
        bigin[:, NM:2 * NM] = astk
        bigin[:, 1536:1536 + 192] = atp
        bigin[:, 1728:1728 + 96] = wct
        bigin[0:75, 1824:1824 + 32] = foldm
        bigin[:, 1856:1856 + 128] = ident
        bigin[:, 1984:1994] = aux
        bigin[0:1, 1994:2002] = aux4
        maps.append(dict(bigin=bigin, zp=zp))
    return maps


_NC_CACHE = None


def _get_nc():
    global _NC_CACHE
    if _NC_CACHE is None:
        _NC_CACHE = _build()
    return _NC_CACHE


def kernel(X, Y, bandwidths, perms):
    nc = _get_nc()
    in_maps = _host_prep(X, Y, bandwidths, perms)
    res = bass_utils.run_bass_kernel_spmd(nc, in_maps, list(range(NC)))
    full = np.zeros((4, 1 + NPER), np.float32)
    full[:, 0] = res.results[0]["out"][:, 0]
    for cid in range(NC):
        full[:, 1 + cid * PPC:1 + (cid + 1) * PPC] = \
            res.results[cid]["out"][:, 1:]
    return full


# revision 38
# speedup vs baseline: 1.3870x; 1.0115x over previous
"""Trainium2 Bass kernel for the 4-kernel MMD permutation test (nn_DUAL_78237124264373).

Math (per core, 25 of the 200 permutations; everything else replicated):
  Z = [X; Y] (768 x 64), d2[r,c] = ||Z_r - Z_c||^2 built on the PE as a single
  rank-66 matmul  d2 = L^T R  with L = [Zt; sq; 1], R = [-2 Zt; 1; sq].
  K0_k = f_k(d2) (symmetric kernel matrix, no diag zeroing).
  With a_p the X-half indicator of permutation p and the zeroed-K statistics
  expressed through symmetric-K0 quantities plus corrections through
  e_j = K0[j, 384+j] (the zeroed stripe), every U_b entry reduces to
     U_b = kap*(q0 - arow0) + W_corr @ e_k + (2/c2)*t + C_k
  where q0 = a K0 a, arow0 = a K0 1 come from one matmul M0 = A_aug K0,
  t is the per-permutation paired-sample sum computed from host-arranged
  Z-row pairs (sentinel rows handle zeroed-stripe pairs), and W_corr folds
  the three correction coefficients into one host-built matrix.

Layout: the four kernels are column-tiled onto PE col-groups, so all
per-permutation statistics live at partition 32*k + p (kernel k, perm p) and
the DVE reductions run once over 128 partitions instead of 4x over 27.
"""

import os
import sys

import numpy as np

if "/opt/trn_rl_repo" not in sys.path:
    sys.path.insert(0, "/opt/trn_rl_repo")

import concourse.bacc as bacc
import concourse.bass as bass
import concourse.mybir as mybir
import concourse.tile as tile
from concourse import bass_utils

N = 384
NM = 768
D = 64
NPER = 200
NC = 8
PPC = NPER // NC  # 25
C1 = float(N * (N - 1))
C2 = float(N * N)
KAP = np.float32(2.0 / C1 + 2.0 / C2)
CB1 = np.float32(1.0 / C1 + 2.0 / C2)
CB2 = np.float32(1.0 / C1)
TCO = np.float32(2.0 / C2)
IC1 = np.float32(1.0 / C1)
IC2 = np.float32(1.0 / C2)
KERNELS = ("gaussian", "laplacian", "gaussian", "laplacian")

F32 = mybir.dt.float32
F32R = mybir.dt.float32r
BF16 = mybir.dt.bfloat16
AF = mybir.ActivationFunctionType
ALU = mybir.AluOpType


def _build():
    nc = bacc.Bacc("TRN2", target_bir_lowering=False, debug=False)
    with tile.TileContext(nc) as tc:
        with tc.tile_pool(name="dram", bufs=1, space="DRAM") as dram, \
             tc.tile_pool(name="io", bufs=1) as io, \
             tc.tile_pool(name="big", bufs=1) as big, \
             tc.tile_pool(name="kpool", bufs=4) as kpool, \
             tc.tile_pool(name="scr", bufs=2) as scr, \
             tc.tile_pool(name="sml", bufs=1) as sml:

            def din(name, shape, dt=F32):
                return dram.tile(shape, dt, kind="ExternalInput", name=name,
                                 uniquify=False)

            # One fused input tensor (single DMA); column layout below.
            W_IN = 2002
            bigin_d = din("bigin", [128, W_IN])
            zp_d = din("zp", [128, 9984 + 192], BF16)  # pair rows + bf16 atp
            out_d = dram.tile([4, 1 + PPC], F32, kind="ExternalOutput",
                              name="out", uniquify=False)

            # ---- phase 0: input DMAs (Zt block first: it gates the PE) ----
            bigin = io.tile([128, W_IN], F32, name="bigin_sb")
            nc.sync.dma_start(out=bigin[:, 0:NM], in_=bigin_d[:, 0:NM])
            nc.sync.dma_start(out=bigin[:, NM:], in_=bigin_d[:, NM:])
            Lbig = bigin[0:D + 1, 0:NM]          # [Zt rows 0-63; ones row 64]
            astk = bigin[:, NM:2 * NM]           # A_aug rows at 32k+i
            atp = bigin[:, 1536:1536 + 192]      # A_aug^T chunks (32-padded)
            wct = bigin[:, 1728:1728 + 96]       # W_corr^T chunks (32-padded)
            fold = bigin[0:75, 1824:1824 + 32]   # 3->1 fold (32-padded)
            ident = bigin[:, 1856:1856 + 128]
            aux = bigin[:, 1984:1994]
            aux4 = bigin[0:1, 1994:2002]
            zpf = io.tile([128, 78 * 128 + 192], BF16, name="zp_sb")
            nc.sync.dma_start(out=zpf[:], in_=zp_d[:])
            zp = zpf[:, 0:9984].rearrange("p (b d) -> p b d", d=128)
            atpb = zpf[:, 9984:9984 + 192]       # A_aug^T chunks in bf16

            ones = io.tile([128, 1], F32, name="ones_sb")
            nc.vector.memset(ones[:], 1.0)

            R_all = io.tile([D + 1, NM], F32, name="R_all")
            # cols 0:4608 = the 6 row-tiles of d2; cols 4608:4686 = the 78
            # pair-distance columns, so ONE wide sqrt covers both.
            d2sb = big.tile([128, 6 * NM + 78], F32, name="d2sb")
            dist = big.tile([128, 6 * NM + 78], F32, name="dist_sb")
            M0sb = big.tile([128, NM], F32, name="M0sb")

            with tc.tile_pool(name="psA", bufs=3, space="PSUM") as psA:
                # ---- phase 1: sq = rowsums of Zt^2, landed at psum
                # partitions 0 (for the sq_col transposes) and 64 (for the
                # R matrix row) via col-tiling ----
                zt2 = scr.tile([D, NM], F32, name="zt2", tag="zt2", bufs=1)
                nc.vector.tensor_tensor(out=zt2[:], in0=Lbig[0:D, :],
                                        in1=Lbig[0:D, :], op=ALU.mult)
                ps_sq = psA.tile([128, NM], F32, name="ps_sq", tag="d2")
                for s in (slice(0, 512), slice(512, NM)):
                    nc.tensor.matmul(ps_sq[0:1, s], ones[0:D, 0:1], zt2[:, s],
                                     start=True, stop=True,
                                     skip_group_check=True)
                    nc.tensor.matmul(ps_sq[D:D + 1, s], ones[0:D, 0:1],
                                     zt2[:, s], start=True, stop=True,
                                     tile_position=(0, D),
                                     skip_group_check=True)
                sqrow = sml.tile([1, NM], F32, name="sqrow")
                nc.vector.tensor_copy(sqrow[:], ps_sq[0:1, :])
                # R = [-2 Zt; sq]; row 64 copies within partition 64.
                nc.vector.tensor_scalar_mul(R_all[0:D, :], Lbig[0:D, :], -2.0)
                nc.vector.tensor_copy(R_all[D:D + 1, :], ps_sq[D:D + 1, :])
                # sq as columns (for the relu bias): 6 tiny PE transposes
                ps_sqc = psA.tile([128, 8], F32, name="ps_sqc", tag="sqc",
                                  bufs=1)
                for r in range(6):
                    nc.tensor.matmul(ps_sqc[:, r:r + 1],
                                     sqrow[0:1, 128 * r:128 * (r + 1)],
                                     ones[0:1, 0:1], is_transpose=True,
                                     start=True, stop=True,
                                     skip_group_check=True)
                sqc = sml.tile([128, 8], F32, name="sqc")
                nc.vector.tensor_copy(sqc[:, 0:6], ps_sqc[:, 0:6])
                # per-gaussian fused bias: (sq[r] + 1e-12) * scale_k
                sqsc = {}
                for k in (0, 2):
                    t = sml.tile([128, 8], F32, name=f"sqsc{k}")
                    nc.vector.tensor_scalar(
                        out=t[:, 0:6], in0=sqc[:, 0:6],
                        scalar1=aux[:, 2 * k:2 * k + 1],
                        scalar2=aux[:, 2 * k + 1:2 * k + 2],
                        op0=ALU.mult, op1=ALU.add)
                    sqsc[k] = t

                kts = [kpool.tile([128, 6 * NM], BF16, name=f"kt{k}",
                                  tag="kt") for k in range(4)]

                # ---- phase 2: psum = -2 Z Z^T + sq[c].  Per row-tile: the
                # DVE adds sq[r] and clamps into d2sb while the two gaussian
                # kernels exp straight out of PSUM (exp of the tiny negative
                # diagonal values is harmless). The DVE also squeezes the
                # bf16 pair-distance pieces into its matmul-wait gaps. ----
                pdiff = sml.tile([128, 78, 64], BF16, name="pdiff")
                pprod = sml.tile([128, 78, 64], BF16, name="pprod")
                for r in range(6):
                    ps_d2 = psA.tile([128, NM], F32, name=f"ps_d2_{r}",
                                     tag="d2")
                    lhs = Lbig[:, 128 * r:128 * (r + 1)]
                    nc.tensor.matmul(ps_d2[:, 0:512], lhs, R_all[:, 0:512],
                                     start=True, stop=True)
                    nc.tensor.matmul(ps_d2[:, 512:NM], lhs, R_all[:, 512:NM],
                                     start=True, stop=True)
                    sl = slice(NM * r, NM * (r + 1))
                    nc.vector.tensor_scalar(
                        out=d2sb[:, sl], in0=ps_d2[:],
                        scalar1=sqc[:, r:r + 1], scalar2=0.0,
                        op0=ALU.add, op1=ALU.max)
                    for k in (0, 2):
                        nc.scalar.activation(kts[k][:, sl], ps_d2[:], AF.Exp,
                                             scale=aux[:, 2 * k:2 * k + 1],
                                             bias=sqsc[k][:, r:r + 1])
                    j = r if r < 3 else r - 3
                    js = slice(26 * j, 26 * (j + 1))
                    if r < 3:
                        nc.vector.tensor_tensor(out=pdiff[:, js, :],
                                                in0=zp[:, js, 0:64],
                                                in1=zp[:, js, 64:128],
                                                op=ALU.subtract)
                    else:
                        nc.vector.tensor_tensor(out=pprod[:, js, :],
                                                in0=pdiff[:, js, :],
                                                in1=pdiff[:, js, :],
                                                op=ALU.mult)
                for j in range(3):
                    js = slice(26 * j, 26 * (j + 1))
                    nc.vector.tensor_reduce(
                        d2sb[:, 6 * NM + 26 * j:6 * NM + 26 * (j + 1)],
                        pprod[:, js, :], axis=mybir.AxisListType.X,
                        op=ALU.add)

            # ---- phase 4: dist = sqrt(d2 + 1e-12), pair cols included ----
            nc.scalar.activation(dist[:], d2sb[:], AF.Sqrt, bias=aux[:, 8:9])
            distp = dist[:, 6 * NM:6 * NM + 78]

            arow = sml.tile([128, 1], F32, name="arow")
            colA = sml.tile([128, 1], F32, name="colA")
            q0c = sml.tile([128, 1], F32, name="q0c")

            with tc.tile_pool(name="psB", bufs=1, space="PSUM") as psB, \
                 tc.tile_pool(name="psC", bufs=1, space="PSUM") as psC:
                # ---- phase 5: laplacian K tiles; M0 = A_aug K0 col-tiled so
                # kernel k's rows land at partitions 32k+i ----
                ps_m = psB.tile([128, NM], F32, name="ps_m")
                for k in (1, 3):
                    for h in range(2):
                        hs = slice(3 * NM * h, 3 * NM * (h + 1))
                        nc.scalar.activation(kts[k][:, hs], dist[:, hs],
                                             AF.Exp,
                                             scale=aux[:, 2 * k:2 * k + 1],
                                             bias=aux[:, 2 * k + 1:2 * k + 2])
                for c in range(6):
                    lhs = atpb[:, 32 * c:32 * (c + 1)]
                    for k in range(4):
                        pr = slice(32 * k, 32 * k + 32)
                        nc.tensor.matmul(ps_m[pr, 0:512], lhs,
                                         kts[k][:, NM * c:NM * c + 512],
                                         start=(c == 0), stop=(c == 5),
                                         tile_position=(0, 32 * k),
                                         skip_group_check=True)
                        nc.tensor.matmul(ps_m[pr, 512:NM], lhs,
                                         kts[k][:, NM * c + 512:NM * (c + 1)],
                                         start=(c == 0), stop=(c == 5),
                                         tile_position=(0, 32 * k),
                                         skip_group_check=True)
                # row stats: copy+rowsum fused, first-half sum, masked q0
                nc.vector.tensor_scalar(
                    out=M0sb[:], in0=ps_m[:], scalar1=1.0, scalar2=0.0,
                    op0=ALU.mult, op1=ALU.add, accum_out=arow[:])
                sA = scr.tile([128, N], F32, name="sA", tag="sA")
                nc.vector.tensor_scalar(
                    out=sA[:], in0=M0sb[:, 0:N], scalar1=1.0, scalar2=0.0,
                    op0=ALU.mult, op1=ALU.add, accum_out=colA[:])
                sB = scr.tile([128, NM], F32, name="sB", tag="sB")
                nc.vector.tensor_tensor(out=sB[:], in0=M0sb[:], in1=astk[:],
                                        op=ALU.mult)
                nc.vector.tensor_reduce(q0c[:], sB[:],
                                        axis=mybir.AxisListType.X, op=ALU.add)

                # ---- pair-term exps (Exp table is already loaded) ----
                # t_k via column-sum matmul then a fold matmul into
                # partitions 32k+p
                d2p = d2sb[:, 6 * NM:6 * NM + 78]
                ps_t = psC.tile([75, 4], F32, name="ps_t", tag="sm", bufs=3)
                expks = []
                for k, kern in enumerate(KERNELS):
                    psrc = d2p if kern == "gaussian" else distp
                    expk = scr.tile([128, 78], F32, name=f"expk{k}",
                                    tag="expk", bufs=4)
                    nc.scalar.activation(expk[:], psrc, AF.Exp,
                                         scale=aux[:, 2 * k:2 * k + 1],
                                         bias=aux[:, 2 * k + 1:2 * k + 2])
                    expks.append(expk)
                    nc.tensor.matmul(ps_t[:, k:k + 1], expk[:, 0:75],
                                     ones[:, 0:1], start=True, stop=True)
                t75s = sml.tile([75, 4], F32, name="t75s")
                nc.vector.tensor_copy(t75s[:], ps_t[:])
                ps_tc = psC.tile([128, 1], F32, name="ps_tc", tag="sm", bufs=3)
                for k in range(4):
                    nc.tensor.matmul(ps_tc[32 * k:32 * k + 32, 0:1], fold[:],
                                     t75s[:, k:k + 1], start=True, stop=True,
                                     tile_position=(0, 32 * k),
                                     skip_group_check=True)
                tcol = sml.tile([128, 1], F32, name="tcol")
                nc.vector.tensor_scalar_mul(tcol[:], ps_tc[:], float(TCO))

                # ---- phase 6: corrections (col-tiled) and stripe sums ----
                ps_corr = psC.tile([128, 1], F32, name="ps_corr", tag="sm",
                                   bufs=3)
                for c in range(3):
                    for k in range(4):
                        nc.tensor.matmul(
                            ps_corr[32 * k:32 * k + 32, 0:1],
                            wct[:, 32 * c:32 * (c + 1)],
                            expks[k][:, 75 + c:76 + c],
                            start=(c == 0), stop=(c == 2),
                            tile_position=(0, 32 * k),
                            skip_group_check=True)
                sesum = sml.tile([3, 4], F32, name="sesum")
                for k in range(4):
                    ps_sek = psC.tile([3, 1], F32, name=f"ps_se{k}", tag="se",
                                      bufs=2)
                    nc.tensor.matmul(ps_sek[:], expks[k][:, 75:78],
                                     ones[:, 0:1], start=True, stop=True)
                    nc.vector.tensor_copy(sesum[:, k:k + 1], ps_sek[:])

                # ---- phase 7: U_b assembly in the stacked [128,1] layout ----
                colB = sml.tile([128, 1], F32, name="colB")
                nc.vector.tensor_tensor(out=colB[:], in0=arow[:], in1=colA[:],
                                        op=ALU.subtract)
                ubv = sml.tile([128, 1], F32, name="ubv")
                nc.vector.tensor_tensor(out=ubv[:], in0=q0c[:], in1=arow[:],
                                        op=ALU.subtract)
                nc.vector.tensor_scalar_mul(ubv[:], ubv[:], float(KAP))
                nc.vector.tensor_tensor(out=ubv[:], in0=ubv[:], in1=ps_corr[:],
                                        op=ALU.add)
                nc.vector.tensor_tensor(out=ubv[:], in0=ubv[:], in1=tcol[:],
                                        op=ALU.add)
                # ---- phase 8: fold everything into one partition-0 row ----
                # frow: [0:128)=ub, [128:256)=colA^T, [256:384)=colB^T,
                # [384:396)=sesum
                frow = sml.tile([1, 396], F32, name="frow")
                nc.sync.dma_start(out=frow[0:1, 0:128], in_=ubv[:])
                nc.sync.dma_start(out=frow[0:1, 128:256], in_=colA[:])
                nc.sync.dma_start(out=frow[0:1, 256:384], in_=colB[:])
                nc.sync.dma_start(out=frow[0:1, 384:396], in_=sesum[:])

                def fr(base, step=32, count=4):
                    ap = frow[0:1, base:base + 1]
                    return bass.AP(ap.tensor, ap.offset,
                                   [ap.ap[0], [step, count]])

                XXv = fr(128 + 25)
                YXv = fr(128 + 26)
                XY0v = fr(256 + 25)
                YYv = fr(256 + 26)
                # se_k = sum_c sesum[4c+k]
                sev = sml.tile([1, 4], F32, name="sev")
                nc.vector.tensor_reduce(
                    sev[:],
                    frow[0:1, 384:396].rearrange("o (c k) -> o k c", k=4),
                    axis=mybir.AxisListType.X, op=ALU.add)
                s0t = sml.tile([1, 4], F32, name="s0t")
                nc.vector.tensor_tensor(out=s0t[:], in0=XXv, in1=YXv,
                                        op=ALU.add)
                nc.vector.tensor_tensor(out=s0t[:], in0=s0t[:], in1=XY0v,
                                        op=ALU.add)
                nc.vector.tensor_tensor(out=s0t[:], in0=s0t[:], in1=YYv,
                                        op=ALU.add)
                ck = sml.tile([1, 4], F32, name="ck")
                nc.vector.tensor_tensor(out=ck[:], in0=s0t[:], in1=sev[:],
                                        op=ALU.subtract)
                nc.vector.tensor_tensor(out=ck[:], in0=ck[:],
                                        in1=aux4[0:1, 0:4], op=ALU.subtract)
                nc.vector.tensor_scalar_mul(ck[:], ck[:], float(IC1))
                u1 = sml.tile([1, 4], F32, name="u1")
                nc.vector.tensor_tensor(out=u1[:], in0=XXv, in1=YYv,
                                        op=ALU.add)
                nc.vector.tensor_tensor(out=u1[:], in0=u1[:],
                                        in1=aux4[0:1, 0:4], op=ALU.subtract)
                nc.vector.tensor_scalar_mul(u1[:], u1[:], float(IC1))
                u2 = sml.tile([1, 4], F32, name="u2")
                nc.vector.tensor_tensor(out=u2[:], in0=XY0v, in1=sev[:],
                                        op=ALU.subtract)
                nc.vector.tensor_scalar_mul(u2[:], u2[:], float(2.0 * IC2))

                # ---- phase 9: contiguous U row + U_b block, two out DMAs ----
                uF = sml.tile([1, 4], F32, name="uF")
                nc.vector.tensor_tensor(out=uF[:], in0=u1[:], in1=u2[:],
                                        op=ALU.subtract)
                ubc = sml.tile([1, 4 * PPC], F32, name="ubc")
                ub_src = frow[0:1, 0:128].rearrange("o (k p) -> o k p", p=32)
                ckap = ck[0:1, 0:4]
                ck_b = bass.AP(ckap.tensor, ckap.offset,
                               [ckap.ap[0], [1, 4], [0, PPC]])
                nc.vector.tensor_tensor(
                    out=ubc[0:1, :].rearrange("o (k p) -> o k p", p=PPC),
                    in0=ub_src[0:1, :, 0:PPC], in1=ck_b, op=ALU.add)
                nc.sync.dma_start(
                    out=out_d[:, 0:1],
                    in_=uF[0:1, :].rearrange("o (k w) -> o k w", w=1))
                nc.sync.dma_start(
                    out=out_d[:, 1:1 + PPC],
                    in_=ubc[0:1, :].rearrange("o (k p) -> o k p", p=PPC))

    nc.compile()
    return nc


def _host_prep(X, Y, bandwidths, perms):
    X = np.ascontiguousarray(X, np.float32)
    Y = np.ascontiguousarray(Y, np.float32)
    perms = np.ascontiguousarray(perms, np.int32)
    Zt = np.zeros((D + 1, NM), np.float32)  # rows 0-63 Zt, row 64 ones
    Zt[0:D] = np.concatenate([X, Y], 0).T
    Zt[D] = 1.0
    b = np.asarray(bandwidths, np.float64)
    gs = (-1.0 / (b * b)).astype(np.float32)
    gb = (gs.astype(np.float64) * 1e-12).astype(np.float32)
    ls = (-1.0 / b).astype(np.float32)
    aux = np.zeros((128, 10), np.float32)
    aux[:, 8] = 1e-12
    d0c = np.zeros(4, np.float64)
    for k, kern in enumerate(KERNELS):
        if kern == "gaussian":
            aux[:, 2 * k] = gs[k]
            aux[:, 2 * k + 1] = gb[k]
            d0c[k] = np.exp(-1e-12 / (b[k] * b[k]))
        else:
            aux[:, 2 * k] = ls[k]
            aux[:, 2 * k + 1] = 0.0
            d0c[k] = np.exp(-np.sqrt(1e-12) / b[k])
    aux4 = np.zeros((1, 8), np.float32)
    aux4[0, 0:4] = (768.0 * d0c).astype(np.float32)
    ident = np.eye(128, dtype=np.float32)
    foldm = np.zeros((75, 32), np.float32)
    foldm[:, :PPC] = (np.arange(75)[:, None] // 3 ==
                      np.arange(PPC)[None, :])

    maps = []
    for cid in range(NC):
        pm = perms[cid * PPC:(cid + 1) * PPC]
        A = np.zeros((27, NM), np.float32)
        A[np.arange(PPC)[:, None], pm[:, :N]] = 1
        A[25, :N] = 1
        A[26, N:] = 1
        astk = np.zeros((128, NM), np.float32)
        for k in range(4):
            astk[32 * k:32 * k + 27] = A
        atp = np.zeros((128, 6 * 32), np.float32)
        for c in range(6):
            atp[:, 32 * c:32 * c + 27] = A[:, 128 * c:128 * (c + 1)].T
        A1 = A[:PPC, :N]
        A2 = A[:PPC, N:]
        Wc = (-KAP * (A1 * A2) + CB1 * A1 + CB2 * A2).astype(np.float32)
        wct = np.zeros((128, 3 * 32), np.float32)
        for c in range(3):
            wct[:, 32 * c:32 * c + PPC] = Wc[:, 128 * c:128 * (c + 1)].T
        pX = pm[:, :N].astype(np.int64).ravel()
        pY = pm[:, N:].astype(np.int64).ravel()
        # Pair-arranged Z rows: [zx | zy] per pair; stripe pairs (pY==pX+384)
        # get a sentinel row with huge distance so f_k -> 0 (matches the
        # zeroed K stripe). Rows 9600..9983 are the stripe-diagonal pairs
        # (they produce the e_k correction vectors).
        Zf = np.concatenate([X, Y], 0)
        zx = Zf[pX]
        zy = Zf[pY]
        stripe = pY == pX + N
        zx[stripe] = 0.0
        zy[stripe] = 0.0
        zx[stripe, 0] = 1e6  # d2=1e12: exp(-1e12/b^2)=exp(-1e6/b)=0
        j = np.arange(N)
        zp = np.concatenate([
            np.concatenate([zx, zy], 1),
            np.concatenate([Zf[j], Zf[N + j]], 1),
        ], 0)
        import ml_dtypes
        zp = zp.reshape(78, 128, 128).transpose(1, 0, 2).reshape(128, 9984)
        zp = np.concatenate([zp, atp], 1).astype(ml_dtypes.bfloat16)
        bigin = np.zeros((128, 2002), np.float32)
        bigin[0:D + 1, 0:NM] = Zt
        bigin[:, NM:2 * NM] = astk
        bigin[:, 1536:1536 + 192] = atp
        bigin[:, 1728:1728 + 96] = wct
        bigin[0:75, 1824:1824 + 32] = foldm
        bigin[:, 1856:1856 + 128] = ident
        bigin[:, 1984:1994] = aux
        bigin[0:1, 1994:2002] = aux4
        maps.append(dict(bigin=bigin, zp=zp))
    return maps


_NC_CACHE = None


def _get_nc():
    global _NC_CACHE
    if _NC_CACHE is None:
        _NC_CACHE = _build()
    return _NC_CACHE


def kernel(X, Y, bandwidths, perms):
    nc = _get_nc()
    in_maps = _host_prep(X, Y, bandwidths, perms)
    res = bass_utils.run_bass_kernel_spmd(nc, in_maps, list(range(NC)))
    full = np.zeros((4, 1 + NPER), np.float32)
    full[:, 0] = res.results[0]["out"][:, 0]
    for cid in range(NC):
        full[:, 1 + cid * PPC:1 + (cid + 1) * PPC] = \
            res.results[cid]["out"][:, 1:]
    return full
